# revision 14
# baseline (speedup 1.0000x reference)
"""Trainium2 Bass kernel for a 2-layer GraphSAGE GNN (ExplainableGNN).

Reference math (eval mode):
    h1 = relu(mean_agg(x) @ W1_l.T + b1 + x @ W1_r.T)
    h2 = relu(mean_agg(h1) @ W2_l.T + b2 + h1 @ W2_r.T)
    out = log_softmax(h2 @ W_lin.T + b_lin)
with mean_agg(v)[i] = sum_{e: dst[e]=i} v[src[e]] / max(indeg[i], 1).

Distribution (8 NeuronCores), v2:
  - Node ownership is INTERLEAVED: node n (group g = n//S, offset r = n%S)
    is owned by core k = r//CHUNK with local row g*CHUNK + (r - k*CHUNK),
    CHUNK = SPAD/8.  This makes each per-group ReduceScatter hand every
    core a contiguous 1568-row piece of its own rows, so the RS for dst
    group g can be issued as soon as group g's partial aggregates are
    scattered - overlapping all collectives with the remaining gather.
  - Edges are sharded by OWNER OF SRC; each core's gather table (y1 =
    x_own @ W1_l.T for layer 1, h1_own for layer 2) is a local 12.7K-row
    table, so int16 gather indices fit.
  - Each core computes partial destination aggregates for ALL N nodes
    (group-padded layout [8*SPAD, H]); per-group ReduceScatter (add)
    hands each core the full sums for its owned rows.
  - Weights are replicated.  Self path (x @ W1_r + b1) stays in SBUF.

Per-core segment-sum machinery: destinations are ordered by
(dst-group, per-core-indegree desc); batches of 128 nodes are padded to
a common per-batch slot count (max over cores, so one SPMD program fits
all cores).  dma_gather fetches message rows (256B each) slot-major, DVE
adds reduce the slot blocks, and dma_scatter_add writes the per-batch
accumulator rows into the zero-initialized partial tensor at their
group-padded row (unique per call -> race free).
"""
import os
import sys

sys.path.insert(0, "/opt/trn_rl_repo")

import numpy as np

import concourse.bass as bass
import concourse.bacc as bacc
import concourse.tile as tile
import concourse.mybir as mybir
from concourse import bass_utils
from concourse.masks import make_identity

P = 128
N_CORES = 8
DIN = 128
H = 64
H2 = 32
NOUT = 2
CH_MAX = int(os.environ.get("KCHMAX", "8192"))
                     # max gather slots buffered per chunk tile
CALL_MAX = 1024      # max idxs per dma_gather/dma_scatter_add call
                     # (SWDGE descriptor ring holds 1024 descriptors)
STAGE_B = 8          # batches per scatter call (8*128 = 1024 idxs)
BBLK = 8             # tiles per batched DMA block in phase A / post loops

F32 = mybir.dt.float32
BF16 = mybir.dt.bfloat16
I16 = mybir.dt.int16
I32 = mybir.dt.int32

# stash of the last run's profiling info (for test harness)
LAST_INFO = {}


# ----------------------------------------------------------------------------
# host-side structure building (pure index bookkeeping)
# ----------------------------------------------------------------------------

def _geom(n_nodes):
    N = n_nodes
    S = N // N_CORES
    TB = -(-S // P)
    SPAD = TB * P
    CHUNK = SPAD // N_CORES
    NLOC = N_CORES * CHUNK           # == SPAD
    TBL = NLOC + P                   # local table rows (+1 zero batch)
    return N, S, TB, SPAD, CHUNK, NLOC, TBL


def _owner_loc(src, S, CHUNK):
    g = src // S
    r = src - g * S
    k = r // CHUNK
    loc = g * CHUNK + (r - k * CHUNK)
    return k, loc


def _owned_nodes(k, N, S, CHUNK, NLOC):
    """Global node id per local row (-1 for dead rows)."""
    own = np.full(NLOC, -1, np.int64)
    for g in range(N_CORES):
        c = np.arange(CHUNK)
        r = k * CHUNK + c
        valid = r < S
        own[g * CHUNK + c[valid]] = g * S + r[valid]
    return own


def _build_structure(src, dst, n_nodes):
    """Common (core-uniform) structure + per-core index streams."""
    N, S, TB, SPAD, CHUNK, NLOC, TBL = _geom(n_nodes)

    deg_global = np.bincount(dst, minlength=N).astype(np.int64)

    k_of, loc_of = _owner_loc(src, S, CHUNK)
    per_core = []
    deg_sorted_all = np.zeros((N_CORES, N_CORES, SPAD), np.int64)
    for k in range(N_CORES):
        m = k_of == k
        src_k = loc_of[m].astype(np.int64)
        dst_k = dst[m].astype(np.int64)
        deg_k = np.bincount(dst_k, minlength=N)
        # CSR by dst
        eorder = np.argsort(dst_k, kind="stable")
        src_csr = src_k[eorder].astype(np.int16)
        indptr = np.zeros(N + 1, np.int64)
        indptr[1:] = np.cumsum(deg_k)
        # per-group ordering by per-core degree (desc), ghosts (-1) trailing
        order = np.full((N_CORES, SPAD), -1, np.int64)
        for g in range(N_CORES):
            dg = deg_k[g * S:(g + 1) * S]
            o = np.argsort(-dg, kind="stable") + g * S
            order[g, :S] = o
            deg_sorted_all[k, g, :S] = deg_k[o]
        per_core.append(dict(deg_k=deg_k, src_csr=src_csr, indptr=indptr,
                             order=order))

    # common per-batch slot counts: max over cores of batch-max degree
    # (desc sort => batch max is its first element)
    s_arr = deg_sorted_all[:, :, ::P].max(axis=0)      # [groups, TB]
    assert s_arr.shape == (N_CORES, TB)

    # chunks: consecutive global batches, <= CH_MAX slots
    chunks = []          # (b0, b1, nslots, col_off)
    b0, cur, coff = 0, 0, 0
    NBATCH = N_CORES * TB
    sflat = s_arr.reshape(-1)
    for b in range(NBATCH):
        w = int(sflat[b]) * P
        assert w <= CH_MAX, f"batch {b} slots {w} exceed CH_MAX"
        if cur + w > CH_MAX and cur > 0:
            chunks.append((b0, b, cur, coff))
            coff += cur // 16
            b0, cur = b, 0
        cur += w
    chunks.append((b0, NBATCH, cur, coff))
    gidx_cols = coff + cur // 16

    # stages: per group, groups of STAGE_B batches
    stages = []          # (g, i0, i1, col_off)
    scoff = 0
    for g in range(N_CORES):
        for i0 in range(0, TB, STAGE_B):
            i1 = min(i0 + STAGE_B, TB)
            stages.append((g, i0, i1, scoff))
            scoff += (i1 - i0) * P // 16
    sidx_cols = scoff

    common = dict(N=N, S=S, TB=TB, SPAD=SPAD, CHUNK=CHUNK, NLOC=NLOC,
                  TBL=TBL, GHOSTS=SPAD - S,
                  s_arr=s_arr, chunks=chunks, stages=stages,
                  gidx_cols=gidx_cols, sidx_cols=sidx_cols,
                  slot_tot=int(sflat.sum()) * P)
    return common, per_core, deg_global


def _wrap16(arr):
    """flat int16 idx array -> [128, n/16] wrapped+replicated layout."""
    n = arr.shape[0]
    assert n % 16 == 0
    w = arr.reshape(-1, 16).T          # [16, n/16]
    return np.tile(w, (8, 1))          # [128, n/16]


def _build_core_inputs(common, pc, deg_global, k, x, weights):
    """Per-core input tensors (index streams + sliced features)."""
    N, S, TB = common["N"], common["S"], common["TB"]
    CHUNK, NLOC, TBL = common["CHUNK"], common["NLOC"], common["TBL"]
    s_arr = common["s_arr"]
    DUMMY = np.int16(NLOC)             # first row of the zero batch

    deg_k = pc["deg_k"]
    src_csr = pc["src_csr"]
    indptr = pc["indptr"]
    order = pc["order"]

    # gather idx stream, chunk-wrapped
    blocks = []
    for g in range(N_CORES):
        for i in range(TB):
            s = int(s_arr[g, i])
            if s == 0:
                continue
            nodes = order[g, i * P:(i + 1) * P]           # [-1 for ghosts]
            valid = nodes >= 0
            nsafe = np.where(valid, nodes, 0)
            degs = np.where(valid, deg_k[nsafe], 0)       # [128]
            base = indptr[nsafe]                          # [128]
            J = np.arange(s)[:, None]                     # [s, 1]
            take = J < degs[None, :]
            pos = np.where(take, base[None, :] + J, 0)
            blk = np.where(take, src_csr[pos], DUMMY).astype(np.int16)
            blocks.append(blk.reshape(-1))                # slot-major (j, p)
    flat = np.concatenate(blocks) if blocks else np.zeros(0, np.int16)
    assert flat.shape[0] == common["slot_tot"]
    # wrap per dma_gather call (CALL_MAX-slot units within each chunk)
    gparts = []
    off = 0
    for (b0, b1, nslots, coff) in common["chunks"]:
        for q0 in range(0, nslots, CALL_MAX):
            qn = min(CALL_MAX, nslots - q0)
            gparts.append(_wrap16(flat[off:off + qn]))
            off += qn
    gidx = (np.concatenate(gparts, axis=1) if gparts
            else np.zeros((P, 0), np.int16))
    assert gidx.shape == (P, common["gidx_cols"])

    # scatter idx stream, stage-wrapped (row within group: 0..S)
    sparts = []
    for (g, i0, i1, scoff) in common["stages"]:
        nodes = order[g, i0 * P:i1 * P]
        loc = np.where(nodes >= 0, nodes - g * S, -1).astype(np.int16)
        sparts.append(_wrap16(loc))
    sidx = np.concatenate(sparts, axis=1)
    assert sidx.shape == (P, common["sidx_cols"])

    # degree (global) of owned rows, tiled [128, TB]
    own = _owned_nodes(k, N, S, CHUNK, NLOC)
    dpad = np.ones(NLOC, np.int32)
    v = own >= 0
    dpad[v] = deg_global[own[v]]
    deg_t = dpad.reshape(TB, P).T.copy()                  # [128, TB]

    # x of owned rows, transposed + padded (zero for dead rows + zero batch)
    xt = np.zeros((DIN, TBL), np.float32)
    xt[:, np.nonzero(v)[0]] = x[own[v]].T

    W1_l, b1, W1_r, W2_l, b2, W2_r, W_lin, b_lin = weights
    w1 = np.concatenate([W1_l.T, W1_r.T], axis=1).astype(np.float32)  # [DIN, 2H]
    w2a = W2_l.T.astype(np.float32).copy()                # [H, H2]
    w2b = W2_r.T.astype(np.float32).copy()                # [H, H2]
    wlin = np.concatenate([W_lin.T, b_lin[None, :]], axis=0).astype(np.float32)
    b1b = np.tile(b1[None, :], (P, 1)).astype(np.float32)  # [128, H]
    b2c = b2[:, None].astype(np.float32).copy()            # [H2, 1]

    return dict(xt=xt, gidx=gidx, sidx=sidx, deg=deg_t, w1=w1,
                w2a=w2a, w2b=w2b, wlin=wlin, b1b=b1b, b2c=b2c)


# ----------------------------------------------------------------------------
# bass program
# ----------------------------------------------------------------------------

def _emit_gather_phase(nc, tc, pools, common, table, partial, rs, gidx_d,
                       sidx_t, tag, groups, rep=0, nqueues=1):
    """gather slot messages from `table`, reduce per batch, scatter-add the
    per-node sums into `partial` (zero-initialized, group-padded rows), and
    issue the per-group ReduceScatter into `rs` as each group completes."""
    S, TB, SPAD = common["S"], common["TB"], common["SPAD"]
    CHUNK, GHOSTS = common["CHUNK"], common["GHOSTS"]
    s_arr = common["s_arr"]
    chunks, stages = common["chunks"], common["stages"]
    gpool, cpool, spool = pools["gather"], pools["cidx"], pools["stage"]

    # map global batch -> (stage index, slot)
    stage_of = {}
    for si, (g, i0, i1, scoff) in enumerate(stages):
        for i in range(i0, i1):
            stage_of[g * TB + i] = (si, i - i0)

    stage_tiles = {}
    sflat = s_arr.reshape(-1)
    qrr = 0

    def emit_rs(g):
        nc.gpsimd.collective_compute(
            "ReduceScatter", mybir.AluOpType.add,
            replica_groups=groups,
            ins=[partial[g * SPAD:(g + 1) * SPAD, :].opt()],
            outs=[rs[g * CHUNK:(g + 1) * CHUNK, :].opt()])

    def flush_stage(si):
        g, i0, i1, scoff = stages[si]
        nb = i1 - i0
        st = stage_tiles.pop(si)
        n_idx = nb * P
        is_final = i1 == TB
        n_real = n_idx - (GHOSTS if is_final else 0)
        nc.gpsimd.dma_scatter_add(
            out_ap=partial[g * SPAD:g * SPAD + S, :],
            in_ap=st[:, :nb, :],
            idxs_ap=sidx_t[:, scoff:scoff + n_idx // 16],
            num_idxs=n_idx,
            num_idxs_reg=n_real,
            elem_size=H,
            queue_num=flush_stage.qrr % nqueues,
        )
        flush_stage.qrr += 1
        if is_final:
            # group g fully scattered on every core (SPMD); defer its RS
            # by one group so the gpsimd-side wait on the scatter DMA sems
            # is already satisfied when the collective issues (no stall)
            if g > 0:
                emit_rs(g - 1)
            if g == N_CORES - 1:
                emit_rs(g)

    flush_stage.qrr = 0
    for (b0, b1, nslots, coff) in chunks:
        if nslots:
            ncols = nslots // 16
            cidx = cpool.tile([P, CH_MAX // 16], I16, tag=f"cidx{tag}")
            nc.sync.dma_start(out=cidx[:, :ncols],
                              in_=gidx_d[:, coff:coff + ncols])
            ch = gpool.tile([P, CH_MAX // P, H], F32, tag=f"ch{tag}")
            # the SWDGE ring caps one call at CALL_MAX descriptors; split
            # the chunk into calls landing in disjoint column ranges
            for q0 in range(0, nslots, CALL_MAX):
                qn = min(CALL_MAX, nslots - q0)
                nc.gpsimd.dma_gather(
                    out_ap=ch[:, q0 // P:(q0 + qn) // P, :],
                    in_ap=table[:],
                    idxs_ap=cidx[:, q0 // 16:(q0 + qn) // 16],
                    num_idxs=qn,
                    num_idxs_reg=qn,
                    elem_size=H,
                    queue_num=qrr % nqueues,
                )
                qrr += 1
        col = 0
        b = b0
        while b < b1:
            si, sl = stage_of[b]
            if si not in stage_tiles:
                stage_tiles[si] = spool.tile([P, STAGE_B, H], F32,
                                             tag=f"st{tag}",
                                             name=f"st{tag}_{si}_r{rep}")
            st = stage_tiles[si]
            s = int(sflat[b])
            # run of consecutive batches with equal s within this stage
            r = 1
            while (b + r < b1 and int(sflat[b + r]) == s
                   and stage_of[b + r] == (si, sl + r)):
                r += 1
            dst_ap = st[:, sl:sl + r, :]
            if s == 0:
                nc.vector.memset(dst_ap, 0.0)
            elif s == 1:
                nc.scalar.activation(dst_ap, ch[:, col:col + r, :],
                                     mybir.ActivationFunctionType.Copy)
            else:
                # one DVE instruction per run: innermost-axis reduction
                # over the slot dim of a permuted view
                view = ch[:, col:col + r * s, :].rearrange(
                    "p (r s) d -> p r d s", s=s)
                nc.vector.tensor_reduce(out=dst_ap, in_=view,
                                        axis=mybir.AxisListType.X,
                                        op=mybir.AluOpType.add)
            col += r * s
            b += r
            # flush once the stage's last batch is done
            g2, _i0, i1_2, _sc = stages[si]
            if b == g2 * TB + i1_2:
                flush_stage(si)
    assert not stage_tiles, f"unflushed stages: {list(stage_tiles)}"


def _build_program(common):
    # KSTAGE bisection: 1=phaseA only, 2=+L1 gather+RS, 4=+L1 post,
    # 5=+L2 gather+RS, 6=full (default)
    KSTAGE = int(os.environ.get("KSTAGE", "6"))
    N, S, TB = common["N"], common["S"], common["TB"]
    SPAD, CHUNK = common["SPAD"], common["CHUNK"]
    NLOC, TBL = common["NLOC"], common["TBL"]
    TB2 = TBL // P
    NPG = N_CORES * SPAD

    KQUEUES = int(os.environ.get("KQUEUES", "4"))
    KSCRATCH = int(os.environ.get("KSCRATCH", "16384"))
    nc = bacc.Bacc("TRN2", target_bir_lowering=False, debug=False,
                   num_devices=N_CORES, num_swdge_queues=KQUEUES,
                   dynamic_dma_scratch_size=KSCRATCH)

    # I/O
    xt_d = nc.dram_tensor("xt", [DIN, TBL], F32, kind="ExternalInput")
    gidx_d = nc.dram_tensor("gidx", [P, common["gidx_cols"]], I16,
                            kind="ExternalInput")
    sidx_d = nc.dram_tensor("sidx", [P, common["sidx_cols"]], I16,
                            kind="ExternalInput")
    deg_d = nc.dram_tensor("deg", [P, TB], I32, kind="ExternalInput")
    w1_d = nc.dram_tensor("w1", [DIN, 2 * H], F32, kind="ExternalInput")
    w2a_d = nc.dram_tensor("w2a", [H, H2], F32, kind="ExternalInput")
    w2b_d = nc.dram_tensor("w2b", [H, H2], F32, kind="ExternalInput")
    wlin_d = nc.dram_tensor("wlin", [H2 + 1, NOUT], F32, kind="ExternalInput")
    b1b_d = nc.dram_tensor("b1b", [P, H], F32, kind="ExternalInput")
    b2c_d = nc.dram_tensor("b2c", [H2, 1], F32, kind="ExternalInput")
    out_d = nc.dram_tensor("out", [NOUT, NLOC], F32, kind="ExternalOutput")

    # internal DRAM
    y1tab = nc.dram_tensor("y1tab", [TBL, H], F32)
    h1tab = nc.dram_tensor("h1tab", [TBL, H], F32)
    partial1 = nc.dram_tensor("partial1", [NPG, H], F32)
    partial2 = nc.dram_tensor("partial2", [NPG, H], F32)
    rs1 = nc.dram_tensor("rs1", [NLOC, H], F32)
    rs2 = nc.dram_tensor("rs2", [NLOC, H], F32)

    groups = [list(range(N_CORES))]

    with tile.TileContext(nc) as tc:
        with (
            tc.tile_pool(name="const", bufs=1) as kpool,
            tc.tile_pool(name="work", bufs=2) as wpool,
            tc.tile_pool(name="small", bufs=4) as mpool,
            tc.tile_pool(name="gather",
                         bufs=int(os.environ.get("KGBUFS", "2"))) as gpool,
            tc.tile_pool(name="cidx", bufs=2) as cpool,
            tc.tile_pool(name="stage", bufs=4) as spool,
            tc.tile_pool(name="psA", bufs=2, space="PSUM") as psA,
            tc.tile_pool(name="psT", bufs=2, space="PSUM") as psT,
            tc.tile_pool(name="ps2", bufs=2, space="PSUM") as ps2,
            tc.tile_pool(name="ps3", bufs=2, space="PSUM") as ps3,
        ):
            pools = dict(gather=gpool, cidx=cpool, stage=spool)

            # ---- constants ----
            w1t = kpool.tile([DIN, 2 * H], F32)
            nc.sync.dma_start(out=w1t[:], in_=w1_d[:])
            w2at_f = kpool.tile([H, H2], F32)
            nc.sync.dma_start(out=w2at_f[:], in_=w2a_d[:])
            w2bt_f = kpool.tile([H, H2], F32)
            nc.sync.dma_start(out=w2bt_f[:], in_=w2b_d[:])
            w2at = kpool.tile([H, H2], BF16)
            nc.vector.tensor_copy(out=w2at[:], in_=w2at_f[:])
            w2bt = kpool.tile([H, H2], BF16)
            nc.vector.tensor_copy(out=w2bt[:], in_=w2bt_f[:])
            wlint = kpool.tile([H2 + 1, NOUT], F32)
            nc.sync.dma_start(out=wlint[:], in_=wlin_d[:])
            b1bt = kpool.tile([P, H], F32)
            nc.sync.dma_start(out=b1bt[:], in_=b1b_d[:])
            b2ct = kpool.tile([H2, 1], F32)
            nc.sync.dma_start(out=b2ct[:], in_=b2c_d[:])
            ident = kpool.tile([P, P], F32)
            make_identity(nc, ident[:])
            sidx_t = kpool.tile([P, common["sidx_cols"]], I16)
            nc.sync.dma_start(out=sidx_t[:], in_=sidx_d[:])
            h1T = kpool.tile([H, NLOC], BF16)
            selfbig = kpool.tile([P, TB * H], F32)

            degt = kpool.tile([P, TB], I32)
            nc.sync.dma_start(out=degt[:], in_=deg_d[:])
            dinv = kpool.tile([P, TB], F32)
            nc.vector.tensor_copy(out=dinv[:], in_=degt[:])
            nc.vector.tensor_scalar_max(dinv[:], dinv[:], 1.0)
            nc.vector.reciprocal(out=dinv[:], in_=dinv[:])

            ZB = 16
            assert NPG % ZB == 0
            ztile = kpool.tile([P, ZB * H], F32)
            nc.vector.memset(ztile[:], 0.0)
            zt = kpool.tile([P, H], F32)
            nc.vector.memset(zt[:], 0.0)

            # KREPS: unroll the whole computation to amortize dispatch
            # noise in wall-clock timing (perf experiments only; default 1)
            def zero_partial(part):
                view = part.ap().rearrange("(a b) d -> a (b d)", b=ZB)
                rows = view.shape[0]
                for r0 in range(0, rows, P):
                    r1 = min(r0 + P, rows)
                    nc.sync.dma_start(out=view[r0:r1, :],
                                      in_=ztile[:r1 - r0, :])

            for rep in range(int(os.environ.get("KREPS", "1"))):
                # ---- phase A: y1 = x@W1_l.T -> y1tab;
                #      selfbig = x@W1_r.T + b1 (SBUF-resident) ----
                sc_A, _ = nc.enter_named_scope("phaseA", False)
                for b0 in range(0, TB2, BBLK):
                    nb = min(BBLK, TB2 - b0)
                    xblk = wpool.tile([DIN, BBLK * P], F32, tag="xblk")
                    nc.sync.dma_start(out=xblk[:, :nb * P],
                                      in_=xt_d[:, b0 * P:(b0 + nb) * P])
                    yblk = wpool.tile([P, BBLK, H], F32, tag="yblk")
                    for i in range(nb):
                        t = b0 + i
                        ps = psA.tile([P, 2 * H], F32, tag="psA")
                        nc.tensor.matmul(out=ps[:],
                                         lhsT=xblk[:, i * P:(i + 1) * P],
                                         rhs=w1t[:], start=True, stop=True)
                        nc.scalar.activation(
                            yblk[:, i, :], ps[:, :H],
                            mybir.ActivationFunctionType.Copy)
                        if t < TB:
                            nc.vector.tensor_add(
                                out=selfbig[:, t * H:(t + 1) * H],
                                in0=ps[:, H:], in1=b1bt[:])
                    yv = y1tab[b0 * P:(b0 + nb) * P, :].rearrange(
                        "(t p) d -> p t d", p=P)
                    nc.sync.dma_start(out=yv, in_=yblk[:, :nb, :])
                nc.leave_named_scope("phaseA", sc_A, False)
                zero_partial(partial1)

                # ---- layer 1 aggregate + pipelined RS ----
                if KSTAGE >= 2:
                    sc_g1, _ = nc.enter_named_scope("L1gather", False)
                    _emit_gather_phase(nc, tc, pools, common, y1tab, partial1,
                                       rs1, gidx_d, sidx_t, tag="L1",
                                       groups=groups, rep=rep,
                                       nqueues=KQUEUES)
                    nc.leave_named_scope("L1gather", sc_g1, False)

                # ---- layer 1 post: h1 = relu(rs1*dinv + self) ----
                sc_p1, _ = nc.enter_named_scope("L1post", False)
                nc.sync.dma_start(out=h1tab[NLOC:TBL, :], in_=zt[:])
                for b0 in range(0, TB if KSTAGE >= 4 else 0, BBLK):
                    nb = min(BBLK, TB - b0)
                    rblk = wpool.tile([P, BBLK, H], F32, tag="rblk")
                    rv = rs1[b0 * P:(b0 + nb) * P, :].rearrange(
                        "(t p) d -> p t d", p=P)
                    nc.sync.dma_start(out=rblk[:, :nb, :], in_=rv)
                    hblk = wpool.tile([P, BBLK, H], F32, tag="hblk")
                    for i in range(nb):
                        t = b0 + i
                        nc.vector.scalar_tensor_tensor(
                            out=hblk[:, i, :], in0=rblk[:, i, :],
                            scalar=dinv[:, t:t + 1],
                            in1=selfbig[:, t * H:(t + 1) * H],
                            op0=mybir.AluOpType.mult,
                            op1=mybir.AluOpType.add)
                        nc.vector.tensor_scalar_max(hblk[:, i, :],
                                                    hblk[:, i, :], 0.0)
                        pst = psT.tile([H, P], F32, tag="psT")
                        nc.tensor.transpose(out=pst[:], in_=hblk[:, i, :],
                                            identity=ident[:])
                        nc.scalar.activation(
                            h1T[:, t * P:(t + 1) * P], pst[:],
                            mybir.ActivationFunctionType.Copy)
                    hv = h1tab[b0 * P:(b0 + nb) * P, :].rearrange(
                        "(t p) d -> p t d", p=P)
                    nc.sync.dma_start(out=hv, in_=hblk[:, :nb, :])
                nc.leave_named_scope("L1post", sc_p1, False)
                zero_partial(partial2)

                # ---- layer 2 aggregate + pipelined RS ----
                if KSTAGE >= 5:
                    sc_g2, _ = nc.enter_named_scope("L2gather", False)
                    _emit_gather_phase(nc, tc, pools, common, h1tab, partial2,
                                       rs2, gidx_d, sidx_t, tag="L2",
                                       groups=groups, rep=rep,
                                       nqueues=KQUEUES)
                    nc.leave_named_scope("L2gather", sc_g2, False)

                # ---- layer 2 post + head (per-block softmax + output) ----
                sc_p2, _ = nc.enter_named_scope("L2post", False)
                for b0 in range(0, TB if KSTAGE >= 6 else 0, BBLK):
                    nb = min(BBLK, TB - b0)
                    rblk = wpool.tile([P, BBLK, H], F32, tag="rblk2")
                    rv = rs2[b0 * P:(b0 + nb) * P, :].rearrange(
                        "(t p) d -> p t d", p=P)
                    nc.sync.dma_start(out=rblk[:, :nb, :], in_=rv)
                    zblk = mpool.tile([P, BBLK * NOUT], F32, tag="zblk")
                    for i in range(nb):
                        t = b0 + i
                        a32 = mpool.tile([P, H], F32, tag="a32")
                        nc.vector.tensor_scalar_mul(a32[:], rblk[:, i, :],
                                                    dinv[:, t:t + 1])
                        pst = psT.tile([H, P], F32, tag="psT")
                        nc.tensor.transpose(out=pst[:], in_=a32[:],
                                            identity=ident[:])
                        aggT = mpool.tile([H, P], BF16, tag="aggT")
                        nc.scalar.activation(
                            aggT[:], pst[:],
                            mybir.ActivationFunctionType.Copy)
                        p2 = ps2.tile([H2, P], F32, tag="p2")
                        nc.tensor.matmul(out=p2[:], lhsT=w2at[:], rhs=aggT[:],
                                         start=True, stop=False)
                        nc.tensor.matmul(out=p2[:], lhsT=w2bt[:],
                                         rhs=h1T[:, t * P:(t + 1) * P],
                                         start=False, stop=True)
                        h2T = mpool.tile([H2 + 1, P], F32, tag="h2T")
                        nc.vector.tensor_scalar(
                            out=h2T[:H2, :], in0=p2[:],
                            scalar1=b2ct[:, :1], scalar2=0.0,
                            op0=mybir.AluOpType.add,
                            op1=mybir.AluOpType.max)
                        nc.vector.memset(h2T[H2:H2 + 1, :], 1.0)
                        # logits = h2 @ W_lin.T + b_lin (ones-row folds bias)
                        p3 = ps3.tile([P, NOUT], F32, tag="p3")
                        nc.tensor.matmul(out=p3[:], lhsT=h2T[:], rhs=wlint[:],
                                         start=True, stop=True)
                        nc.vector.tensor_copy(
                            out=zblk[:, i * NOUT:(i + 1) * NOUT], in_=p3[:])
                    # block log-softmax over the 2 classes
                    zv = zblk[:, :nb * NOUT].rearrange("p (t c) -> p t c",
                                                       c=NOUT)
                    mtb = mpool.tile([P, BBLK], F32, tag="mtb")
                    nc.vector.tensor_max(out=mtb[:, :nb], in0=zv[:, :, 0],
                                         in1=zv[:, :, 1])
                    nc.vector.tensor_sub(out=zv[:, :, 0], in0=zv[:, :, 0],
                                         in1=mtb[:, :nb])
                    nc.vector.tensor_sub(out=zv[:, :, 1], in0=zv[:, :, 1],
                                         in1=mtb[:, :nb])
                    eb = mpool.tile([P, BBLK * NOUT], F32, tag="eb")
                    nc.scalar.activation(eb[:, :nb * NOUT],
                                         zblk[:, :nb * NOUT],
                                         mybir.ActivationFunctionType.Exp)
                    ev = eb[:, :nb * NOUT].rearrange("p (t c) -> p t c",
                                                     c=NOUT)
                    stb = mpool.tile([P, BBLK], F32, tag="stb")
                    nc.vector.tensor_add(out=stb[:, :nb], in0=ev[:, :, 0],
                                         in1=ev[:, :, 1])
                    lsb = mpool.tile([P, BBLK], F32, tag="lsb")
                    nc.scalar.activation(lsb[:, :nb], stb[:, :nb],
                                         mybir.ActivationFunctionType.Ln)
                    nc.vector.tensor_sub(out=zv[:, :, 0], in0=zv[:, :, 0],
                                         in1=lsb[:, :nb])
                    nc.vector.tensor_sub(out=zv[:, :, 1], in0=zv[:, :, 1],
                                         in1=lsb[:, :nb])
                    # transpose block result to [2, nb*P] and store
                    rT = wpool.tile([NOUT, BBLK * P], F32, tag="rT")
                    for i in range(nb):
                        pst = psT.tile([H, P], F32, tag="psT")
                        nc.tensor.transpose(
                            out=pst[:NOUT, :],
                            in_=zblk[:, i * NOUT:(i + 1) * NOUT],
                            identity=ident[:])
                        nc.vector.tensor_copy(
                            out=rT[:, i * P:(i + 1) * P],
                            in_=pst[:NOUT, :])
                    nc.sync.dma_start(
                        out=out_d[:, b0 * P:(b0 + nb) * P],
                        in_=rT[:, :nb * P])
                nc.leave_named_scope("L2post", sc_p2, False)

    nc.compile()
    return nc


# ----------------------------------------------------------------------------
# runner: persistent jitted executable (mirrors bass2jax.run_bass_via_pjrt,
# but reusable so repeat executions can be wall-clock timed)
# ----------------------------------------------------------------------------

def make_runner(nc, n_cores=N_CORES):
    import jax
    from jax.sharding import Mesh, PartitionSpec
    from jax.experimental.shard_map import shard_map
    import concourse.mybir as mb
    from concourse import bass2jax

    bass2jax.install_neuronx_cc_hook()
    assert nc.dbg_addr is None
    pname = nc.partition_id_tensor.name if nc.partition_id_tensor else None

    in_names, out_names, out_avals = [], [], []
    for alloc in nc.m.functions[0].allocations:
        if not isinstance(alloc, mb.MemoryLocationSet):
            continue
        name = alloc.memorylocations[0].name
        if alloc.kind == "ExternalInput":
            if name != pname:
                in_names.append(name)
        elif alloc.kind == "ExternalOutput":
            out_names.append(name)
            out_avals.append(jax.core.ShapedArray(
                tuple(alloc.tensor_shape), mb.dt.np(alloc.dtype)))
    n_params = len(in_names)
    all_names = in_names + out_names
    if pname is not None:
        all_names = all_names + [pname]

    def _body(*args):
        operands = list(args)
        if pname is not None:
            operands.append(bass2jax.partition_id_tensor())
        outs = bass2jax._bass_exec_p.bind(
            *operands, out_avals=tuple(out_avals), in_names=tuple(all_names),
            out_names=tuple(out_names), lowering_input_output_aliases=(),
            sim_require_finite=True, sim_require_nnan=True, nc=nc)
        return tuple(outs)

    devices = jax.devices()[:n_cores]
    mesh = Mesh(np.asarray(devices), ("core",))
    n_outs = len(out_names)
    sharded = jax.jit(
        shard_map(_body, mesh=mesh,
                  in_specs=(PartitionSpec("core"),) * (n_params + n_outs),
                  out_specs=(PartitionSpec("core"),) * n_outs,
                  check_rep=False),
        donate_argnums=tuple(range(n_params, n_params + n_outs)),
        keep_unused=True)

    from jax.sharding import NamedSharding
    shard = NamedSharding(mesh, PartitionSpec("core"))

    def prepare(in_maps):
        """Pre-stage the concatenated inputs on the devices."""
        concat_in = [np.concatenate([np.asarray(m[nm]) for m in in_maps],
                                    axis=0) for nm in in_names]
        dev_in = [jax.device_put(a, shard) for a in concat_in]
        jax.block_until_ready(dev_in)
        return dev_in

    def run_prepared(dev_in):
        concat_zeros = [np.zeros((n_cores * a.shape[0], *a.shape[1:]),
                                 a.dtype) for a in out_avals]
        dev_zeros = [jax.device_put(z, shard) for z in concat_zeros]
        jax.block_until_ready(dev_zeros)
        outs = sharded(*dev_in, *dev_zeros)
        return jax.block_until_ready(outs)

    def run(in_maps):
        outs = run_prepared(prepare(in_maps))
        return [
            {nm: np.asarray(outs[i]).reshape(n_cores, *out_avals[i].shape)[c]
             for i, nm in enumerate(out_names)}
            for c in range(n_cores)
        ]

    run.prepare = prepare
    run.run_prepared = run_prepared
    return run


# ----------------------------------------------------------------------------
# entry point
# ----------------------------------------------------------------------------

def kernel(x, edge_index, W1_l, b1_l, W1_r, W2_l, b2_l, W2_r, W_lin, b_lin):
    x = np.ascontiguousarray(np.asarray(x, np.float32))
    ei = np.asarray(edge_index)
    src = ei[0].astype(np.int64)
    dst = ei[1].astype(np.int64)
    n_nodes = x.shape[0]

    weights = (np.asarray(W1_l, np.float32), np.asarray(b1_l, np.float32),
               np.asarray(W1_r, np.float32), np.asarray(W2_l, np.float32),
               np.asarray(b2_l, np.float32), np.asarray(W2_r, np.float32),
               np.asarray(W_lin, np.float32), np.asarray(b_lin, np.float32))

    common, per_core, deg_global = _build_structure(src, dst, n_nodes)
    in_maps = [_build_core_inputs(common, per_core[k], deg_global, k, x,
                                  weights) for k in range(N_CORES)]

    nc = _build_program(common)

    run = make_runner(nc)
    results = run(in_maps)
    LAST_INFO.clear()
    LAST_INFO.update(slot_tot=common["slot_tot"], runner=run,
                     in_maps=in_maps, nc=nc)

    N, S = common["N"], common["S"]
    CHUNK, NLOC = common["CHUNK"], common["NLOC"]
    out = np.zeros((N, NOUT), np.float32)
    for k in range(N_CORES):
        res = results[k]["out"].T                          # [NLOC, NOUT]
        own = _owned_nodes(k, N, S, CHUNK, NLOC)
        v = own >= 0
        out[own[v]] = res[v]
    return out.astype(np.float32)


# revision 16
# speedup vs baseline: 1.0904x; 1.0904x over previous
"""Trainium2 Bass kernel for a 2-layer GraphSAGE GNN (ExplainableGNN).

Reference math (eval mode):
    h1 = relu(mean_agg(x) @ W1_l.T + b1 + x @ W1_r.T)
    h2 = relu(mean_agg(h1) @ W2_l.T + b2 + h1 @ W2_r.T)
    out = log_softmax(h2 @ W_lin.T + b_lin)
with mean_agg(v)[i] = sum_{e: dst[e]=i} v[src[e]] / max(indeg[i], 1).

Distribution (8 NeuronCores), v2:
  - Node ownership is INTERLEAVED: node n (group g = n//S, offset r = n%S)
    is owned by core k = r//CHUNK with local row g*CHUNK + (r - k*CHUNK),
    CHUNK = SPAD/8.  This makes each per-group ReduceScatter hand every
    core a contiguous 1568-row piece of its own rows, so the RS for dst
    group g can be issued as soon as group g's partial aggregates are
    scattered - overlapping all collectives with the remaining gather.
  - Edges are sharded by OWNER OF SRC; each core's gather table (y1 =
    x_own @ W1_l.T for layer 1, h1_own for layer 2) is a local 12.7K-row
    table, so int16 gather indices fit.
  - Each core computes partial destination aggregates for ALL N nodes
    (group-padded layout [8*SPAD, H]); per-group ReduceScatter (add)
    hands each core the full sums for its owned rows.
  - Weights are replicated.  Self path (x @ W1_r + b1) stays in SBUF.

Per-core segment-sum machinery: destinations are ordered by
(dst-group, per-core-indegree desc); batches of 128 nodes are padded to
a common per-batch slot count (max over cores, so one SPMD program fits
all cores).  dma_gather fetches message rows (256B each) slot-major, DVE
adds reduce the slot blocks, and dma_scatter_add writes the per-batch
accumulator rows into the zero-initialized partial tensor at their
group-padded row (unique per call -> race free).
"""
import os
import sys

sys.path.insert(0, "/opt/trn_rl_repo")

import numpy as np

import concourse.bass as bass
import concourse.bacc as bacc
import concourse.tile as tile
import concourse.mybir as mybir
from concourse import bass_utils
from concourse.masks import make_identity

P = 128
N_CORES = 8
DIN = 128
H = 64
H2 = 32
NOUT = 2
CH_MAX = int(os.environ.get("KCHMAX", "8192"))
                     # max gather slots buffered per chunk tile
CALL_MAX = 1024      # max idxs per dma_gather/dma_scatter_add call
                     # (SWDGE descriptor ring holds 1024 descriptors)
STAGE_B = 8          # batches per scatter call (8*128 = 1024 idxs)
BBLK = 8             # tiles per batched DMA block in phase A / post loops

F32 = mybir.dt.float32
BF16 = mybir.dt.bfloat16
I16 = mybir.dt.int16
I32 = mybir.dt.int32

# stash of the last run's profiling info (for test harness)
LAST_INFO = {}


# ----------------------------------------------------------------------------
# host-side structure building (pure index bookkeeping)
# ----------------------------------------------------------------------------

def _geom(n_nodes):
    N = n_nodes
    S = N // N_CORES
    TB = -(-S // P)
    SPAD = TB * P
    CHUNK = SPAD // N_CORES
    NLOC = N_CORES * CHUNK           # == SPAD
    TBL = NLOC + P                   # local table rows (+1 zero batch)
    return N, S, TB, SPAD, CHUNK, NLOC, TBL


def _owner_loc(src, S, CHUNK):
    g = src // S
    r = src - g * S
    k = r // CHUNK
    loc = g * CHUNK + (r - k * CHUNK)
    return k, loc


def _owned_nodes(k, N, S, CHUNK, NLOC):
    """Global node id per local row (-1 for dead rows)."""
    own = np.full(NLOC, -1, np.int64)
    for g in range(N_CORES):
        c = np.arange(CHUNK)
        r = k * CHUNK + c
        valid = r < S
        own[g * CHUNK + c[valid]] = g * S + r[valid]
    return own


def _build_structure(src, dst, n_nodes):
    """Common (core-uniform) structure + per-core index streams."""
    N, S, TB, SPAD, CHUNK, NLOC, TBL = _geom(n_nodes)

    deg_global = np.bincount(dst, minlength=N).astype(np.int64)

    k_of, loc_of = _owner_loc(src, S, CHUNK)
    per_core = []
    deg_sorted_all = np.zeros((N_CORES, N_CORES, SPAD), np.int64)
    for k in range(N_CORES):
        m = k_of == k
        src_k = loc_of[m].astype(np.int64)
        dst_k = dst[m].astype(np.int64)
        deg_k = np.bincount(dst_k, minlength=N)
        # CSR by dst
        eorder = np.argsort(dst_k, kind="stable")
        src_csr = src_k[eorder].astype(np.int16)
        indptr = np.zeros(N + 1, np.int64)
        indptr[1:] = np.cumsum(deg_k)
        # per-group ordering by per-core degree (desc), ghosts (-1) trailing
        order = np.full((N_CORES, SPAD), -1, np.int64)
        for g in range(N_CORES):
            dg = deg_k[g * S:(g + 1) * S]
            o = np.argsort(-dg, kind="stable") + g * S
            order[g, :S] = o
            deg_sorted_all[k, g, :S] = deg_k[o]
        per_core.append(dict(deg_k=deg_k, src_csr=src_csr, indptr=indptr,
                             order=order))

    # common per-batch slot counts: max over cores of batch-max degree
    # (desc sort => batch max is its first element)
    s_arr = deg_sorted_all[:, :, ::P].max(axis=0)      # [groups, TB]
    assert s_arr.shape == (N_CORES, TB)

    # chunks: consecutive global batches, <= CH_MAX slots
    chunks = []          # (b0, b1, nslots, col_off)
    b0, cur, coff = 0, 0, 0
    NBATCH = N_CORES * TB
    sflat = s_arr.reshape(-1)
    for b in range(NBATCH):
        w = int(sflat[b]) * P
        assert w <= CH_MAX, f"batch {b} slots {w} exceed CH_MAX"
        if cur + w > CH_MAX and cur > 0:
            chunks.append((b0, b, cur, coff))
            coff += cur // 16
            b0, cur = b, 0
        cur += w
    chunks.append((b0, NBATCH, cur, coff))
    gidx_cols = coff + cur // 16

    # stages: per group, groups of STAGE_B batches
    stages = []          # (g, i0, i1, col_off)
    scoff = 0
    for g in range(N_CORES):
        for i0 in range(0, TB, STAGE_B):
            i1 = min(i0 + STAGE_B, TB)
            stages.append((g, i0, i1, scoff))
            scoff += (i1 - i0) * P // 16
    sidx_cols = scoff

    common = dict(N=N, S=S, TB=TB, SPAD=SPAD, CHUNK=CHUNK, NLOC=NLOC,
                  TBL=TBL, GHOSTS=SPAD - S,
                  s_arr=s_arr, chunks=chunks, stages=stages,
                  gidx_cols=gidx_cols, sidx_cols=sidx_cols,
                  slot_tot=int(sflat.sum()) * P)
    return common, per_core, deg_global


def _wrap16(arr):
    """flat int16 idx array -> [128, n/16] wrapped+replicated layout."""
    n = arr.shape[0]
    assert n % 16 == 0
    w = arr.reshape(-1, 16).T          # [16, n/16]
    return np.tile(w, (8, 1))          # [128, n/16]


def _build_core_inputs(common, pc, deg_global, k, x, weights):
    """Per-core input tensors (index streams + sliced features)."""
    N, S, TB = common["N"], common["S"], common["TB"]
    CHUNK, NLOC, TBL = common["CHUNK"], common["NLOC"], common["TBL"]
    s_arr = common["s_arr"]
    DUMMY = np.int16(NLOC)             # first row of the zero batch

    deg_k = pc["deg_k"]
    src_csr = pc["src_csr"]
    indptr = pc["indptr"]
    order = pc["order"]

    # gather idx stream, chunk-wrapped
    blocks = []
    for g in range(N_CORES):
        for i in range(TB):
            s = int(s_arr[g, i])
            if s == 0:
                continue
            nodes = order[g, i * P:(i + 1) * P]           # [-1 for ghosts]
            valid = nodes >= 0
            nsafe = np.where(valid, nodes, 0)
            degs = np.where(valid, deg_k[nsafe], 0)       # [128]
            base = indptr[nsafe]                          # [128]
            J = np.arange(s)[:, None]                     # [s, 1]
            take = J < degs[None, :]
            pos = np.where(take, base[None, :] + J, 0)
            blk = np.where(take, src_csr[pos], DUMMY).astype(np.int16)
            blocks.append(blk.reshape(-1))                # slot-major (j, p)
    flat = np.concatenate(blocks) if blocks else np.zeros(0, np.int16)
    assert flat.shape[0] == common["slot_tot"]
    # wrap per dma_gather call (CALL_MAX-slot units within each chunk)
    gparts = []
    off = 0
    for (b0, b1, nslots, coff) in common["chunks"]:
        for q0 in range(0, nslots, CALL_MAX):
            qn = min(CALL_MAX, nslots - q0)
            gparts.append(_wrap16(flat[off:off + qn]))
            off += qn
    gidx = (np.concatenate(gparts, axis=1) if gparts
            else np.zeros((P, 0), np.int16))
    assert gidx.shape == (P, common["gidx_cols"])

    # scatter idx stream, stage-wrapped (row within group: 0..S)
    sparts = []
    for (g, i0, i1, scoff) in common["stages"]:
        nodes = order[g, i0 * P:i1 * P]
        loc = np.where(nodes >= 0, nodes - g * S, -1).astype(np.int16)
        sparts.append(_wrap16(loc))
    sidx = np.concatenate(sparts, axis=1)
    assert sidx.shape == (P, common["sidx_cols"])

    # degree (global) of owned rows, tiled [128, TB]
    own = _owned_nodes(k, N, S, CHUNK, NLOC)
    dpad = np.ones(NLOC, np.int32)
    v = own >= 0
    dpad[v] = deg_global[own[v]]
    deg_t = dpad.reshape(TB, P).T.copy()                  # [128, TB]

    # x of owned rows, transposed + padded (zero for dead rows + zero batch)
    xt = np.zeros((DIN, TBL), np.float32)
    xt[:, np.nonzero(v)[0]] = x[own[v]].T

    W1_l, b1, W1_r, W2_l, b2, W2_r, W_lin, b_lin = weights
    w1 = np.concatenate([W1_l.T, W1_r.T], axis=1).astype(np.float32)  # [DIN, 2H]
    w2a = W2_l.T.astype(np.float32).copy()                # [H, H2]
    w2b = W2_r.T.astype(np.float32).copy()                # [H, H2]
    wlin = np.concatenate([W_lin.T, b_lin[None, :]], axis=0).astype(np.float32)
    b1b = np.tile(b1[None, :], (P, 1)).astype(np.float32)  # [128, H]
    b2c = b2[:, None].astype(np.float32).copy()            # [H2, 1]

    return dict(xt=xt, gidx=gidx, sidx=sidx, deg=deg_t, w1=w1,
                w2a=w2a, w2b=w2b, wlin=wlin, b1b=b1b, b2c=b2c)


# ----------------------------------------------------------------------------
# bass program
# ----------------------------------------------------------------------------

def _emit_gather_phase(nc, tc, pools, common, table, partial, rs, gidx_d,
                       sidx_t, tag, groups, rep=0, nqueues=1):
    """gather slot messages from `table`, reduce per batch, scatter-add the
    per-node sums into `partial` (zero-initialized, group-padded rows), and
    issue the per-group ReduceScatter into `rs` as each group completes."""
    S, TB, SPAD = common["S"], common["TB"], common["SPAD"]
    CHUNK, GHOSTS = common["CHUNK"], common["GHOSTS"]
    s_arr = common["s_arr"]
    chunks, stages = common["chunks"], common["stages"]
    gpool, cpool, spool = pools["gather"], pools["cidx"], pools["stage"]

    # map global batch -> (stage index, slot)
    stage_of = {}
    for si, (g, i0, i1, scoff) in enumerate(stages):
        for i in range(i0, i1):
            stage_of[g * TB + i] = (si, i - i0)

    stage_tiles = {}
    sflat = s_arr.reshape(-1)
    qrr = 0

    def emit_rs(g):
        nc.gpsimd.collective_compute(
            "ReduceScatter", mybir.AluOpType.add,
            replica_groups=groups,
            ins=[partial[g * SPAD:(g + 1) * SPAD, :].opt()],
            outs=[rs[g * CHUNK:(g + 1) * CHUNK, :].opt()])

    def flush_stage(si):
        g, i0, i1, scoff = stages[si]
        nb = i1 - i0
        st = stage_tiles.pop(si)
        n_idx = nb * P
        is_final = i1 == TB
        n_real = n_idx - (GHOSTS if is_final else 0)
        nc.gpsimd.dma_scatter_add(
            out_ap=partial[g * SPAD:g * SPAD + S, :],
            in_ap=st[:, :nb, :],
            idxs_ap=sidx_t[:, scoff:scoff + n_idx // 16],
            num_idxs=n_idx,
            num_idxs_reg=n_real,
            elem_size=H,
            queue_num=nqueues - 1,
        )
        flush_stage.qrr += 1
        if is_final:
            # group g fully scattered on every core (SPMD); defer its RS
            # by one group so the gpsimd-side wait on the scatter DMA sems
            # is already satisfied when the collective issues (no stall)
            if g > 0:
                emit_rs(g - 1)
            if g == N_CORES - 1:
                emit_rs(g)

    flush_stage.qrr = 0
    for (b0, b1, nslots, coff) in chunks:
        if nslots:
            ncols = nslots // 16
            cidx = cpool.tile([P, CH_MAX // 16], I16, tag=f"cidx{tag}")
            nc.sync.dma_start(out=cidx[:, :ncols],
                              in_=gidx_d[:, coff:coff + ncols])
            ch = gpool.tile([P, CH_MAX // P, H], F32, tag=f"ch{tag}")
            # the SWDGE ring caps one call at CALL_MAX descriptors; split
            # the chunk into calls landing in disjoint column ranges
            for q0 in range(0, nslots, CALL_MAX):
                qn = min(CALL_MAX, nslots - q0)
                nc.gpsimd.dma_gather(
                    out_ap=ch[:, q0 // P:(q0 + qn) // P, :],
                    in_ap=table[:],
                    idxs_ap=cidx[:, q0 // 16:(q0 + qn) // 16],
                    num_idxs=qn,
                    num_idxs_reg=qn,
                    elem_size=H,
                    queue_num=qrr % max(nqueues - 1, 1),
                )
                qrr += 1
        col = 0
        b = b0
        while b < b1:
            si, sl = stage_of[b]
            if si not in stage_tiles:
                stage_tiles[si] = spool.tile([P, STAGE_B, H], F32,
                                             tag=f"st{tag}",
                                             name=f"st{tag}_{si}_r{rep}")
            st = stage_tiles[si]
            s = int(sflat[b])
            # run of consecutive batches with equal s within this stage
            r = 1
            while (b + r < b1 and int(sflat[b + r]) == s
                   and stage_of[b + r] == (si, sl + r)):
                r += 1
            dst_ap = st[:, sl:sl + r, :]
            if s == 0:
                nc.vector.memset(dst_ap, 0.0)
            elif s == 1:
                nc.scalar.activation(dst_ap, ch[:, col:col + r, :],
                                     mybir.ActivationFunctionType.Copy)
            else:
                # one DVE instruction per run: innermost-axis reduction
                # over the slot dim of a permuted view
                view = ch[:, col:col + r * s, :].rearrange(
                    "p (r s) d -> p r d s", s=s)
                nc.vector.tensor_reduce(out=dst_ap, in_=view,
                                        axis=mybir.AxisListType.X,
                                        op=mybir.AluOpType.add)
            col += r * s
            b += r
            # flush once the stage's last batch is done
            g2, _i0, i1_2, _sc = stages[si]
            if b == g2 * TB + i1_2:
                flush_stage(si)
    assert not stage_tiles, f"unflushed stages: {list(stage_tiles)}"


def _build_program(common):
    # KSTAGE bisection: 1=phaseA only, 2=+L1 gather+RS, 4=+L1 post,
    # 5=+L2 gather+RS, 6=full (default)
    KSTAGE = int(os.environ.get("KSTAGE", "6"))
    N, S, TB = common["N"], common["S"], common["TB"]
    SPAD, CHUNK = common["SPAD"], common["CHUNK"]
    NLOC, TBL = common["NLOC"], common["TBL"]
    TB2 = TBL // P
    NPG = N_CORES * SPAD

    KQUEUES = int(os.environ.get("KQUEUES", "4"))
    KSCRATCH = int(os.environ.get("KSCRATCH", "16384"))
    nc = bacc.Bacc("TRN2", target_bir_lowering=False, debug=False,
                   num_devices=N_CORES, num_swdge_queues=KQUEUES,
                   dynamic_dma_scratch_size=KSCRATCH)

    # I/O
    xt_d = nc.dram_tensor("xt", [DIN, TBL], F32, kind="ExternalInput")
    gidx_d = nc.dram_tensor("gidx", [P, common["gidx_cols"]], I16,
                            kind="ExternalInput")
    sidx_d = nc.dram_tensor("sidx", [P, common["sidx_cols"]], I16,
                            kind="ExternalInput")
    deg_d = nc.dram_tensor("deg", [P, TB], I32, kind="ExternalInput")
    w1_d = nc.dram_tensor("w1", [DIN, 2 * H], F32, kind="ExternalInput")
    w2a_d = nc.dram_tensor("w2a", [H, H2], F32, kind="ExternalInput")
    w2b_d = nc.dram_tensor("w2b", [H, H2], F32, kind="ExternalInput")
    wlin_d = nc.dram_tensor("wlin", [H2 + 1, NOUT], F32, kind="ExternalInput")
    b1b_d = nc.dram_tensor("b1b", [P, H], F32, kind="ExternalInput")
    b2c_d = nc.dram_tensor("b2c", [H2, 1], F32, kind="ExternalInput")
    out_d = nc.dram_tensor("out", [NOUT, NLOC], F32, kind="ExternalOutput")

    # internal DRAM
    y1tab = nc.dram_tensor("y1tab", [TBL, H], F32)
    h1tab = nc.dram_tensor("h1tab", [TBL, H], F32)
    partial1 = nc.dram_tensor("partial1", [NPG, H], F32)
    partial2 = nc.dram_tensor("partial2", [NPG, H], F32)
    rs1 = nc.dram_tensor("rs1", [NLOC, H], F32)
    rs2 = nc.dram_tensor("rs2", [NLOC, H], F32)

    groups = [list(range(N_CORES))]

    with tile.TileContext(nc) as tc:
        with (
            tc.tile_pool(name="const", bufs=1) as kpool,
            tc.tile_pool(name="work", bufs=2) as wpool,
            tc.tile_pool(name="small", bufs=4) as mpool,
            tc.tile_pool(name="gather",
                         bufs=int(os.environ.get("KGBUFS", "2"))) as gpool,
            tc.tile_pool(name="cidx", bufs=2) as cpool,
            tc.tile_pool(name="stage",
                         bufs=int(os.environ.get("KSBUFS", "6"))) as spool,
            tc.tile_pool(name="psA", bufs=2, space="PSUM") as psA,
            tc.tile_pool(name="psT", bufs=2, space="PSUM") as psT,
            tc.tile_pool(name="ps2", bufs=2, space="PSUM") as ps2,
            tc.tile_pool(name="ps3", bufs=2, space="PSUM") as ps3,
        ):
            pools = dict(gather=gpool, cidx=cpool, stage=spool)

            # ---- constants ----
            w1t = kpool.tile([DIN, 2 * H], F32)
            nc.sync.dma_start(out=w1t[:], in_=w1_d[:])
            w2at_f = kpool.tile([H, H2], F32)
            nc.sync.dma_start(out=w2at_f[:], in_=w2a_d[:])
            w2bt_f = kpool.tile([H, H2], F32)
            nc.sync.dma_start(out=w2bt_f[:], in_=w2b_d[:])
            w2at = kpool.tile([H, H2], BF16)
            nc.vector.tensor_copy(out=w2at[:], in_=w2at_f[:])
            w2bt = kpool.tile([H, H2], BF16)
            nc.vector.tensor_copy(out=w2bt[:], in_=w2bt_f[:])
            wlint = kpool.tile([H2 + 1, NOUT], F32)
            nc.sync.dma_start(out=wlint[:], in_=wlin_d[:])
            b1bt = kpool.tile([P, H], F32)
            nc.sync.dma_start(out=b1bt[:], in_=b1b_d[:])
            b2ct = kpool.tile([H2, 1], F32)
            nc.sync.dma_start(out=b2ct[:], in_=b2c_d[:])
            ident = kpool.tile([P, P], F32)
            make_identity(nc, ident[:])
            sidx_t = kpool.tile([P, common["sidx_cols"]], I16)
            nc.sync.dma_start(out=sidx_t[:], in_=sidx_d[:])
            h1T = kpool.tile([H, NLOC], BF16)
            selfbig = kpool.tile([P, TB * H], F32)
            zbig = kpool.tile([P, TB * NOUT], F32)
            ebig = kpool.tile([P, TB * NOUT], F32)
            mt = kpool.tile([P, TB], F32)
            stt = kpool.tile([P, TB], F32)
            lst = kpool.tile([P, TB], F32)

            degt = kpool.tile([P, TB], I32)
            nc.sync.dma_start(out=degt[:], in_=deg_d[:])
            dinv = kpool.tile([P, TB], F32)
            nc.vector.tensor_copy(out=dinv[:], in_=degt[:])
            nc.vector.tensor_scalar_max(dinv[:], dinv[:], 1.0)
            nc.vector.reciprocal(out=dinv[:], in_=dinv[:])

            ZB = 16
            assert NPG % ZB == 0
            ztile = kpool.tile([P, ZB * H], F32)
            nc.vector.memset(ztile[:], 0.0)
            zt = kpool.tile([P, H], F32)
            nc.vector.memset(zt[:], 0.0)

            # KREPS: unroll the whole computation to amortize dispatch
            # noise in wall-clock timing (perf experiments only; default 1)
            def zero_partial(part):
                view = part.ap().rearrange("(a b) d -> a (b d)", b=ZB)
                rows = view.shape[0]
                for r0 in range(0, rows, P):
                    r1 = min(r0 + P, rows)
                    nc.sync.dma_start(out=view[r0:r1, :],
                                      in_=ztile[:r1 - r0, :])

            for rep in range(int(os.environ.get("KREPS", "1"))):
                # ---- phase A: y1 = x@W1_l.T -> y1tab;
                #      selfbig = x@W1_r.T + b1 (SBUF-resident) ----
                sc_A, _ = nc.enter_named_scope("phaseA", False)
                for b0 in range(0, TB2, BBLK):
                    nb = min(BBLK, TB2 - b0)
                    xblk = wpool.tile([DIN, BBLK * P], F32, tag="xblk")
                    nc.sync.dma_start(out=xblk[:, :nb * P],
                                      in_=xt_d[:, b0 * P:(b0 + nb) * P])
                    yblk = wpool.tile([P, BBLK, H], F32, tag="yblk")
                    for i in range(nb):
                        t = b0 + i
                        ps = psA.tile([P, 2 * H], F32, tag="psA")
                        nc.tensor.matmul(out=ps[:],
                                         lhsT=xblk[:, i * P:(i + 1) * P],
                                         rhs=w1t[:], start=True, stop=True)
                        nc.scalar.activation(
                            yblk[:, i, :], ps[:, :H],
                            mybir.ActivationFunctionType.Copy)
                        if t < TB:
                            nc.vector.tensor_add(
                                out=selfbig[:, t * H:(t + 1) * H],
                                in0=ps[:, H:], in1=b1bt[:])
                    yv = y1tab[b0 * P:(b0 + nb) * P, :].rearrange(
                        "(t p) d -> p t d", p=P)
                    nc.sync.dma_start(out=yv, in_=yblk[:, :nb, :])
                nc.leave_named_scope("phaseA", sc_A, False)
                zero_partial(partial1)

                # ---- layer 1 aggregate + pipelined RS ----
                if KSTAGE >= 2:
                    sc_g1, _ = nc.enter_named_scope("L1gather", False)
                    _emit_gather_phase(nc, tc, pools, common, y1tab, partial1,
                                       rs1, gidx_d, sidx_t, tag="L1",
                                       groups=groups, rep=rep,
                                       nqueues=KQUEUES)
                    nc.leave_named_scope("L1gather", sc_g1, False)

                # ---- layer 1 post: h1 = relu(rs1*dinv + self) ----
                sc_p1, _ = nc.enter_named_scope("L1post", False)
                nc.sync.dma_start(out=h1tab[NLOC:TBL, :], in_=zt[:])
                for b0 in range(0, TB if KSTAGE >= 4 else 0, BBLK):
                    nb = min(BBLK, TB - b0)
                    rblk = wpool.tile([P, BBLK, H], F32, tag="rblk")
                    rv = rs1[b0 * P:(b0 + nb) * P, :].rearrange(
                        "(t p) d -> p t d", p=P)
                    nc.sync.dma_start(out=rblk[:, :nb, :], in_=rv)
                    hblk = wpool.tile([P, BBLK, H], F32, tag="hblk")
                    for i in range(nb):
                        t = b0 + i
                        nc.vector.scalar_tensor_tensor(
                            out=hblk[:, i, :], in0=rblk[:, i, :],
                            scalar=dinv[:, t:t + 1],
                            in1=selfbig[:, t * H:(t + 1) * H],
                            op0=mybir.AluOpType.mult,
                            op1=mybir.AluOpType.add)
                        nc.vector.tensor_scalar_max(hblk[:, i, :],
                                                    hblk[:, i, :], 0.0)
                        pst = psT.tile([H, P], F32, tag="psT")
                        nc.tensor.transpose(out=pst[:], in_=hblk[:, i, :],
                                            identity=ident[:])
                        nc.scalar.activation(
                            h1T[:, t * P:(t + 1) * P], pst[:],
                            mybir.ActivationFunctionType.Copy)
                    hv = h1tab[b0 * P:(b0 + nb) * P, :].rearrange(
                        "(t p) d -> p t d", p=P)
                    nc.sync.dma_start(out=hv, in_=hblk[:, :nb, :])
                nc.leave_named_scope("L1post", sc_p1, False)
                zero_partial(partial2)

                # ---- layer 2 aggregate + pipelined RS ----
                if KSTAGE >= 5:
                    sc_g2, _ = nc.enter_named_scope("L2gather", False)
                    _emit_gather_phase(nc, tc, pools, common, h1tab, partial2,
                                       rs2, gidx_d, sidx_t, tag="L2",
                                       groups=groups, rep=rep,
                                       nqueues=KQUEUES)
                    nc.leave_named_scope("L2gather", sc_g2, False)

                # ---- layer 2 post + head ----
                sc_p2, _ = nc.enter_named_scope("L2post", False)
                for b0 in range(0, TB if KSTAGE >= 6 else 0, BBLK):
                    nb = min(BBLK, TB - b0)
                    rblk = wpool.tile([P, BBLK, H], F32, tag="rblk2")
                    rv = rs2[b0 * P:(b0 + nb) * P, :].rearrange(
                        "(t p) d -> p t d", p=P)
                    nc.sync.dma_start(out=rblk[:, :nb, :], in_=rv)
                    for i in range(nb):
                        t = b0 + i
                        a32 = mpool.tile([P, H], F32, tag="a32")
                        nc.vector.tensor_scalar_mul(a32[:], rblk[:, i, :],
                                                    dinv[:, t:t + 1])
                        pst = psT.tile([H, P], F32, tag="psT")
                        nc.tensor.transpose(out=pst[:], in_=a32[:],
                                            identity=ident[:])
                        aggT = mpool.tile([H, P], BF16, tag="aggT")
                        nc.scalar.activation(
                            aggT[:], pst[:],
                            mybir.ActivationFunctionType.Copy)
                        p2 = ps2.tile([H2, P], F32, tag="p2")
                        nc.tensor.matmul(out=p2[:], lhsT=w2at[:], rhs=aggT[:],
                                         start=True, stop=False)
                        nc.tensor.matmul(out=p2[:], lhsT=w2bt[:],
                                         rhs=h1T[:, t * P:(t + 1) * P],
                                         start=False, stop=True)
                        h2T = mpool.tile([H2 + 1, P], F32, tag="h2T")
                        nc.vector.tensor_scalar(
                            out=h2T[:H2, :], in0=p2[:],
                            scalar1=b2ct[:, :1], scalar2=0.0,
                            op0=mybir.AluOpType.add,
                            op1=mybir.AluOpType.max)
                        nc.vector.memset(h2T[H2:H2 + 1, :], 1.0)
                        # logits = h2 @ W_lin.T + b_lin (ones-row folds bias)
                        p3 = ps3.tile([P, NOUT], F32, tag="p3")
                        nc.tensor.matmul(out=p3[:], lhsT=h2T[:], rhs=wlint[:],
                                         start=True, stop=True)
                        nc.scalar.activation(
                            zbig[:, t * NOUT:(t + 1) * NOUT], p3[:],
                            mybir.ActivationFunctionType.Copy)
                if KSTAGE >= 6:
                    # ---- batched log-softmax over the 2 classes ----
                    zv = zbig[:].rearrange("p (t c) -> p t c", c=NOUT)
                    ev = ebig[:].rearrange("p (t c) -> p t c", c=NOUT)
                    nc.vector.tensor_max(out=mt[:], in0=zv[:, :, 0],
                                         in1=zv[:, :, 1])
                    nc.vector.tensor_sub(out=zv[:, :, 0], in0=zv[:, :, 0],
                                         in1=mt[:])
                    nc.vector.tensor_sub(out=zv[:, :, 1], in0=zv[:, :, 1],
                                         in1=mt[:])
                    nc.scalar.activation(ebig[:], zbig[:],
                                         mybir.ActivationFunctionType.Exp)
                    nc.vector.tensor_add(out=stt[:], in0=ev[:, :, 0],
                                         in1=ev[:, :, 1])
                    nc.scalar.activation(lst[:], stt[:],
                                         mybir.ActivationFunctionType.Ln)
                    nc.vector.tensor_sub(out=zv[:, :, 0], in0=zv[:, :, 0],
                                         in1=lst[:])
                    nc.vector.tensor_sub(out=zv[:, :, 1], in0=zv[:, :, 1],
                                         in1=lst[:])
                    # ---- transpose out to [2, NLOC] and store ----
                    for b0 in range(0, TB, BBLK):
                        nb = min(BBLK, TB - b0)
                        rT = wpool.tile([NOUT, BBLK * P], F32, tag="rT")
                        for i in range(nb):
                            t = b0 + i
                            pst = psT.tile([H, P], F32, tag="psT")
                            nc.tensor.transpose(
                                out=pst[:NOUT, :],
                                in_=zbig[:, t * NOUT:(t + 1) * NOUT],
                                identity=ident[:])
                            nc.vector.tensor_copy(
                                out=rT[:, i * P:(i + 1) * P],
                                in_=pst[:NOUT, :])
                        nc.sync.dma_start(
                            out=out_d[:, b0 * P:(b0 + nb) * P],
                            in_=rT[:, :nb * P])
                nc.leave_named_scope("L2post", sc_p2, False)

    nc.compile()
    return nc


# ----------------------------------------------------------------------------
# runner: persistent jitted executable (mirrors bass2jax.run_bass_via_pjrt,
# but reusable so repeat executions can be wall-clock timed)
# ----------------------------------------------------------------------------

def make_runner(nc, n_cores=N_CORES):
    import jax
    from jax.sharding import Mesh, PartitionSpec
    from jax.experimental.shard_map import shard_map
    import concourse.mybir as mb
    from concourse import bass2jax

    bass2jax.install_neuronx_cc_hook()
    assert nc.dbg_addr is None
    pname = nc.partition_id_tensor.name if nc.partition_id_tensor else None

    in_names, out_names, out_avals = [], [], []
    for alloc in nc.m.functions[0].allocations:
        if not isinstance(alloc, mb.MemoryLocationSet):
            continue
        name = alloc.memorylocations[0].name
        if alloc.kind == "ExternalInput":
            if name != pname:
                in_names.append(name)
        elif alloc.kind == "ExternalOutput":
            out_names.append(name)
            out_avals.append(jax.core.ShapedArray(
                tuple(alloc.tensor_shape), mb.dt.np(alloc.dtype)))
    n_params = len(in_names)
    all_names = in_names + out_names
    if pname is not None:
        all_names = all_names + [pname]

    def _body(*args):
        operands = list(args)
        if pname is not None:
            operands.append(bass2jax.partition_id_tensor())
        outs = bass2jax._bass_exec_p.bind(
            *operands, out_avals=tuple(out_avals), in_names=tuple(all_names),
            out_names=tuple(out_names), lowering_input_output_aliases=(),
            sim_require_finite=True, sim_require_nnan=True, nc=nc)
        return tuple(outs)

    devices = jax.devices()[:n_cores]
    mesh = Mesh(np.asarray(devices), ("core",))
    n_outs = len(out_names)
    sharded = jax.jit(
        shard_map(_body, mesh=mesh,
                  in_specs=(PartitionSpec("core"),) * (n_params + n_outs),
                  out_specs=(PartitionSpec("core"),) * n_outs,
                  check_rep=False),
        donate_argnums=tuple(range(n_params, n_params + n_outs)),
        keep_unused=True)

    from jax.sharding import NamedSharding
    shard = NamedSharding(mesh, PartitionSpec("core"))

    def prepare(in_maps):
        """Pre-stage the concatenated inputs on the devices."""
        concat_in = [np.concatenate([np.asarray(m[nm]) for m in in_maps],
                                    axis=0) for nm in in_names]
        dev_in = [jax.device_put(a, shard) for a in concat_in]
        jax.block_until_ready(dev_in)
        return dev_in

    def run_prepared(dev_in):
        concat_zeros = [np.zeros((n_cores * a.shape[0], *a.shape[1:]),
                                 a.dtype) for a in out_avals]
        dev_zeros = [jax.device_put(z, shard) for z in concat_zeros]
        jax.block_until_ready(dev_zeros)
        outs = sharded(*dev_in, *dev_zeros)
        return jax.block_until_ready(outs)

    def run(in_maps):
        outs = run_prepared(prepare(in_maps))
        return [
            {nm: np.asarray(outs[i]).reshape(n_cores, *out_avals[i].shape)[c]
             for i, nm in enumerate(out_names)}
            for c in range(n_cores)
        ]

    run.prepare = prepare
    run.run_prepared = run_prepared
    return run


# ----------------------------------------------------------------------------
# entry point
# ----------------------------------------------------------------------------

def kernel(x, edge_index, W1_l, b1_l, W1_r, W2_l, b2_l, W2_r, W_lin, b_lin):
    x = np.ascontiguousarray(np.asarray(x, np.float32))
    ei = np.asarray(edge_index)
    src = ei[0].astype(np.int64)
    dst = ei[1].astype(np.int64)
    n_nodes = x.shape[0]

    weights = (np.asarray(W1_l, np.float32), np.asarray(b1_l, np.float32),
               np.asarray(W1_r, np.float32), np.asarray(W2_l, np.float32),
               np.asarray(b2_l, np.float32), np.asarray(W2_r, np.float32),
               np.asarray(W_lin, np.float32), np.asarray(b_lin, np.float32))

    common, per_core, deg_global = _build_structure(src, dst, n_nodes)
    in_maps = [_build_core_inputs(common, per_core[k], deg_global, k, x,
                                  weights) for k in range(N_CORES)]

    nc = _build_program(common)

    run = make_runner(nc)
    results = run(in_maps)
    LAST_INFO.clear()
    LAST_INFO.update(slot_tot=common["slot_tot"], runner=run,
                     in_maps=in_maps, nc=nc)

    N, S = common["N"], common["S"]
    CHUNK, NLOC = common["CHUNK"], common["NLOC"]
    out = np.zeros((N, NOUT), np.float32)
    for k in range(N_CORES):
        res = results[k]["out"].T                          # [NLOC, NOUT]
        own = _owned_nodes(k, N, S, CHUNK, NLOC)
        v = own >= 0
        out[own[v]] = res[v]
    return out.astype(np.float32)


# revision 17
# speedup vs baseline: 1.0928x; 1.0023x over previous
"""Trainium2 Bass kernel for a 2-layer GraphSAGE GNN (ExplainableGNN).

Reference math (eval mode):
    h1 = relu(mean_agg(x) @ W1_l.T + b1 + x @ W1_r.T)
    h2 = relu(mean_agg(h1) @ W2_l.T + b2 + h1 @ W2_r.T)
    out = log_softmax(h2 @ W_lin.T + b_lin)
with mean_agg(v)[i] = sum_{e: dst[e]=i} v[src[e]] / max(indeg[i], 1).

Distribution (8 NeuronCores), v2:
  - Node ownership is INTERLEAVED: node n (group g = n//S, offset r = n%S)
    is owned by core k = r//CHUNK with local row g*CHUNK + (r - k*CHUNK),
    CHUNK = SPAD/8.  This makes each per-group ReduceScatter hand every
    core a contiguous 1568-row piece of its own rows, so the RS for dst
    group g can be issued as soon as group g's partial aggregates are
    scattered - overlapping all collectives with the remaining gather.
  - Edges are sharded by OWNER OF SRC; each core's gather table (y1 =
    x_own @ W1_l.T for layer 1, h1_own for layer 2) is a local 12.7K-row
    table, so int16 gather indices fit.
  - Each core computes partial destination aggregates for ALL N nodes
    (group-padded layout [8*SPAD, H]); per-group ReduceScatter (add)
    hands each core the full sums for its owned rows.
  - Weights are replicated.  Self path (x @ W1_r + b1) stays in SBUF.

Per-core segment-sum machinery: destinations are ordered by
(dst-group, per-core-indegree desc); batches of 128 nodes are padded to
a common per-batch slot count (max over cores, so one SPMD program fits
all cores).  dma_gather fetches message rows (256B each) slot-major, DVE
adds reduce the slot blocks, and dma_scatter_add writes the per-batch
accumulator rows into the zero-initialized partial tensor at their
group-padded row (unique per call -> race free).
"""
import os
import sys

sys.path.insert(0, "/opt/trn_rl_repo")

import numpy as np

import concourse.bass as bass
import concourse.bacc as bacc
import concourse.tile as tile
import concourse.mybir as mybir
from concourse import bass_utils
from concourse.masks import make_identity

P = 128
N_CORES = 8
DIN = 128
H = 64
H2 = 32
NOUT = 2
CH_MAX = int(os.environ.get("KCHMAX", "8192"))
                     # max gather slots buffered per chunk tile
CALL_MAX = 1024      # max idxs per dma_gather/dma_scatter_add call
                     # (SWDGE descriptor ring holds 1024 descriptors)
STAGE_B = 8          # batches per scatter call (8*128 = 1024 idxs)
BBLK = 8             # tiles per batched DMA block in phase A / post loops

F32 = mybir.dt.float32
BF16 = mybir.dt.bfloat16
I16 = mybir.dt.int16
I32 = mybir.dt.int32

# stash of the last run's profiling info (for test harness)
LAST_INFO = {}


# ----------------------------------------------------------------------------
# host-side structure building (pure index bookkeeping)
# ----------------------------------------------------------------------------

def _geom(n_nodes):
    N = n_nodes
    S = N // N_CORES
    TB = -(-S // P)
    SPAD = TB * P
    CHUNK = SPAD // N_CORES
    NLOC = N_CORES * CHUNK           # == SPAD
    TBL = NLOC + P                   # local table rows (+1 zero batch)
    return N, S, TB, SPAD, CHUNK, NLOC, TBL


def _owner_loc(src, S, CHUNK):
    g = src // S
    r = src - g * S
    k = r // CHUNK
    loc = g * CHUNK + (r - k * CHUNK)
    return k, loc


def _owned_nodes(k, N, S, CHUNK, NLOC):
    """Global node id per local row (-1 for dead rows)."""
    own = np.full(NLOC, -1, np.int64)
    for g in range(N_CORES):
        c = np.arange(CHUNK)
        r = k * CHUNK + c
        valid = r < S
        own[g * CHUNK + c[valid]] = g * S + r[valid]
    return own


def _build_structure(src, dst, n_nodes):
    """Common (core-uniform) structure + per-core index streams."""
    N, S, TB, SPAD, CHUNK, NLOC, TBL = _geom(n_nodes)

    deg_global = np.bincount(dst, minlength=N).astype(np.int64)

    k_of, loc_of = _owner_loc(src, S, CHUNK)
    per_core = []
    deg_sorted_all = np.zeros((N_CORES, N_CORES, SPAD), np.int64)
    for k in range(N_CORES):
        m = k_of == k
        src_k = loc_of[m].astype(np.int64)
        dst_k = dst[m].astype(np.int64)
        deg_k = np.bincount(dst_k, minlength=N)
        # CSR by dst
        eorder = np.argsort(dst_k, kind="stable")
        src_csr = src_k[eorder].astype(np.int16)
        indptr = np.zeros(N + 1, np.int64)
        indptr[1:] = np.cumsum(deg_k)
        # per-group ordering by per-core degree (desc), ghosts (-1) trailing
        order = np.full((N_CORES, SPAD), -1, np.int64)
        for g in range(N_CORES):
            dg = deg_k[g * S:(g + 1) * S]
            o = np.argsort(-dg, kind="stable") + g * S
            order[g, :S] = o
            deg_sorted_all[k, g, :S] = deg_k[o]
        per_core.append(dict(deg_k=deg_k, src_csr=src_csr, indptr=indptr,
                             order=order))

    # common per-batch slot counts: max over cores of batch-max degree
    # (desc sort => batch max is its first element)
    s_arr = deg_sorted_all[:, :, ::P].max(axis=0)      # [groups, TB]
    assert s_arr.shape == (N_CORES, TB)

    # chunks: consecutive global batches, <= CH_MAX slots
    chunks = []          # (b0, b1, nslots, col_off)
    b0, cur, coff = 0, 0, 0
    NBATCH = N_CORES * TB
    sflat = s_arr.reshape(-1)
    for b in range(NBATCH):
        w = int(sflat[b]) * P
        assert w <= CH_MAX, f"batch {b} slots {w} exceed CH_MAX"
        if cur + w > CH_MAX and cur > 0:
            chunks.append((b0, b, cur, coff))
            coff += cur // 16
            b0, cur = b, 0
        cur += w
    chunks.append((b0, NBATCH, cur, coff))
    gidx_cols = coff + cur // 16

    # stages: per group, groups of STAGE_B batches
    stages = []          # (g, i0, i1, col_off)
    scoff = 0
    for g in range(N_CORES):
        for i0 in range(0, TB, STAGE_B):
            i1 = min(i0 + STAGE_B, TB)
            stages.append((g, i0, i1, scoff))
            scoff += (i1 - i0) * P // 16
    sidx_cols = scoff

    common = dict(N=N, S=S, TB=TB, SPAD=SPAD, CHUNK=CHUNK, NLOC=NLOC,
                  TBL=TBL, GHOSTS=SPAD - S,
                  s_arr=s_arr, chunks=chunks, stages=stages,
                  gidx_cols=gidx_cols, sidx_cols=sidx_cols,
                  slot_tot=int(sflat.sum()) * P)
    return common, per_core, deg_global


def _wrap16(arr):
    """flat int16 idx array -> [128, n/16] wrapped+replicated layout."""
    n = arr.shape[0]
    assert n % 16 == 0
    w = arr.reshape(-1, 16).T          # [16, n/16]
    return np.tile(w, (8, 1))          # [128, n/16]


def _build_core_inputs(common, pc, deg_global, k, x, weights):
    """Per-core input tensors (index streams + sliced features)."""
    N, S, TB = common["N"], common["S"], common["TB"]
    CHUNK, NLOC, TBL = common["CHUNK"], common["NLOC"], common["TBL"]
    s_arr = common["s_arr"]
    DUMMY = np.int16(NLOC)             # first row of the zero batch

    deg_k = pc["deg_k"]
    src_csr = pc["src_csr"]
    indptr = pc["indptr"]
    order = pc["order"]

    # gather idx stream, chunk-wrapped
    blocks = []
    for g in range(N_CORES):
        for i in range(TB):
            s = int(s_arr[g, i])
            if s == 0:
                continue
            nodes = order[g, i * P:(i + 1) * P]           # [-1 for ghosts]
            valid = nodes >= 0
            nsafe = np.where(valid, nodes, 0)
            degs = np.where(valid, deg_k[nsafe], 0)       # [128]
            base = indptr[nsafe]                          # [128]
            J = np.arange(s)[:, None]                     # [s, 1]
            take = J < degs[None, :]
            pos = np.where(take, base[None, :] + J, 0)
            blk = np.where(take, src_csr[pos], DUMMY).astype(np.int16)
            blocks.append(blk.reshape(-1))                # slot-major (j, p)
    flat = np.concatenate(blocks) if blocks else np.zeros(0, np.int16)
    assert flat.shape[0] == common["slot_tot"]
    # wrap per dma_gather call (CALL_MAX-slot units within each chunk)
    gparts = []
    off = 0
    for (b0, b1, nslots, coff) in common["chunks"]:
        for q0 in range(0, nslots, CALL_MAX):
            qn = min(CALL_MAX, nslots - q0)
            gparts.append(_wrap16(flat[off:off + qn]))
            off += qn
    gidx = (np.concatenate(gparts, axis=1) if gparts
            else np.zeros((P, 0), np.int16))
    assert gidx.shape == (P, common["gidx_cols"])

    # scatter idx stream, stage-wrapped (row within group: 0..S)
    sparts = []
    for (g, i0, i1, scoff) in common["stages"]:
        nodes = order[g, i0 * P:i1 * P]
        loc = np.where(nodes >= 0, nodes - g * S, -1).astype(np.int16)
        sparts.append(_wrap16(loc))
    sidx = np.concatenate(sparts, axis=1)
    assert sidx.shape == (P, common["sidx_cols"])

    # degree (global) of owned rows, tiled [128, TB]
    own = _owned_nodes(k, N, S, CHUNK, NLOC)
    dpad = np.ones(NLOC, np.int32)
    v = own >= 0
    dpad[v] = deg_global[own[v]]
    deg_t = dpad.reshape(TB, P).T.copy()                  # [128, TB]

    # x of owned rows, transposed + padded (zero for dead rows + zero batch)
    xt = np.zeros((DIN, TBL), np.float32)
    xt[:, np.nonzero(v)[0]] = x[own[v]].T

    W1_l, b1, W1_r, W2_l, b2, W2_r, W_lin, b_lin = weights
    w1 = np.concatenate([W1_l.T, W1_r.T], axis=1).astype(np.float32)  # [DIN, 2H]
    w2a = W2_l.T.astype(np.float32).copy()                # [H, H2]
    w2b = W2_r.T.astype(np.float32).copy()                # [H, H2]
    wlin = np.concatenate([W_lin.T, b_lin[None, :]], axis=0).astype(np.float32)
    b1b = np.tile(b1[None, :], (P, 1)).astype(np.float32)  # [128, H]
    b2c = b2[:, None].astype(np.float32).copy()            # [H2, 1]

    return dict(xt=xt, gidx=gidx, sidx=sidx, deg=deg_t, w1=w1,
                w2a=w2a, w2b=w2b, wlin=wlin, b1b=b1b, b2c=b2c)


# ----------------------------------------------------------------------------
# bass program
# ----------------------------------------------------------------------------

def _emit_gather_phase(nc, tc, pools, common, table, partial, rs, gidx_d,
                       sidx_t, tag, groups, rep=0, nqueues=1):
    """gather slot messages from `table`, reduce per batch, scatter-add the
    per-node sums into `partial` (zero-initialized, group-padded rows), and
    issue the per-group ReduceScatter into `rs` as each group completes."""
    S, TB, SPAD = common["S"], common["TB"], common["SPAD"]
    CHUNK, GHOSTS = common["CHUNK"], common["GHOSTS"]
    s_arr = common["s_arr"]
    chunks, stages = common["chunks"], common["stages"]
    gpool, cpool, spool = pools["gather"], pools["cidx"], pools["stage"]

    # map global batch -> (stage index, slot)
    stage_of = {}
    for si, (g, i0, i1, scoff) in enumerate(stages):
        for i in range(i0, i1):
            stage_of[g * TB + i] = (si, i - i0)

    stage_tiles = {}
    sflat = s_arr.reshape(-1)
    qrr = 0

    def emit_rs(g):
        nc.gpsimd.collective_compute(
            "ReduceScatter", mybir.AluOpType.add,
            replica_groups=groups,
            ins=[partial[g * SPAD:(g + 1) * SPAD, :].opt()],
            outs=[rs[g * CHUNK:(g + 1) * CHUNK, :].opt()])

    def flush_stage(si):
        g, i0, i1, scoff = stages[si]
        nb = i1 - i0
        st = stage_tiles.pop(si)
        n_idx = nb * P
        is_final = i1 == TB
        n_real = n_idx - (GHOSTS if is_final else 0)
        nc.gpsimd.dma_scatter_add(
            out_ap=partial[g * SPAD:g * SPAD + S, :],
            in_ap=st[:, :nb, :],
            idxs_ap=sidx_t[:, scoff:scoff + n_idx // 16],
            num_idxs=n_idx,
            num_idxs_reg=n_real,
            elem_size=H,
            queue_num=flush_stage.qrr % nqueues,
        )
        flush_stage.qrr += 1
        if is_final:
            # group g fully scattered on every core (SPMD); defer its RS
            # by one group so the gpsimd-side wait on the scatter DMA sems
            # is already satisfied when the collective issues (no stall)
            if g > 0:
                emit_rs(g - 1)
            if g == N_CORES - 1:
                emit_rs(g)

    flush_stage.qrr = 0
    for (b0, b1, nslots, coff) in chunks:
        if nslots:
            ncols = nslots // 16
            cidx = cpool.tile([P, CH_MAX // 16], I16, tag=f"cidx{tag}")
            nc.sync.dma_start(out=cidx[:, :ncols],
                              in_=gidx_d[:, coff:coff + ncols])
            ch = gpool.tile([P, CH_MAX // P, H], F32, tag=f"ch{tag}")
            # the SWDGE ring caps one call at CALL_MAX descriptors; split
            # the chunk into calls landing in disjoint column ranges
            for q0 in range(0, nslots, CALL_MAX):
                qn = min(CALL_MAX, nslots - q0)
                nc.gpsimd.dma_gather(
                    out_ap=ch[:, q0 // P:(q0 + qn) // P, :],
                    in_ap=table[:],
                    idxs_ap=cidx[:, q0 // 16:(q0 + qn) // 16],
                    num_idxs=qn,
                    num_idxs_reg=qn,
                    elem_size=H,
                    queue_num=qrr % nqueues,
                )
                qrr += 1
        col = 0
        b = b0
        while b < b1:
            si, sl = stage_of[b]
            if si not in stage_tiles:
                stage_tiles[si] = spool.tile([P, STAGE_B, H], F32,
                                             tag=f"st{tag}",
                                             name=f"st{tag}_{si}_r{rep}")
            st = stage_tiles[si]
            s = int(sflat[b])
            # run of consecutive batches with equal s within this stage
            r = 1
            while (b + r < b1 and int(sflat[b + r]) == s
                   and stage_of[b + r] == (si, sl + r)):
                r += 1
            dst_ap = st[:, sl:sl + r, :]
            if s == 0:
                nc.vector.memset(dst_ap, 0.0)
            elif s == 1:
                nc.scalar.activation(dst_ap, ch[:, col:col + r, :],
                                     mybir.ActivationFunctionType.Copy)
            else:
                # one DVE instruction per run: innermost-axis reduction
                # over the slot dim of a permuted view
                view = ch[:, col:col + r * s, :].rearrange(
                    "p (r s) d -> p r d s", s=s)
                nc.vector.tensor_reduce(out=dst_ap, in_=view,
                                        axis=mybir.AxisListType.X,
                                        op=mybir.AluOpType.add)
            col += r * s
            b += r
            # flush once the stage's last batch is done
            g2, _i0, i1_2, _sc = stages[si]
            if b == g2 * TB + i1_2:
                flush_stage(si)
    assert not stage_tiles, f"unflushed stages: {list(stage_tiles)}"


def _build_program(common):
    # KSTAGE bisection: 1=phaseA only, 2=+L1 gather+RS, 4=+L1 post,
    # 5=+L2 gather+RS, 6=full (default)
    KSTAGE = int(os.environ.get("KSTAGE", "6"))
    N, S, TB = common["N"], common["S"], common["TB"]
    SPAD, CHUNK = common["SPAD"], common["CHUNK"]
    NLOC, TBL = common["NLOC"], common["TBL"]
    TB2 = TBL // P
    NPG = N_CORES * SPAD

    KQUEUES = int(os.environ.get("KQUEUES", "4"))
    KSCRATCH = int(os.environ.get("KSCRATCH", "16384"))
    nc = bacc.Bacc("TRN2", target_bir_lowering=False, debug=False,
                   num_devices=N_CORES, num_swdge_queues=KQUEUES,
                   dynamic_dma_scratch_size=KSCRATCH)

    # I/O
    xt_d = nc.dram_tensor("xt", [DIN, TBL], F32, kind="ExternalInput")
    gidx_d = nc.dram_tensor("gidx", [P, common["gidx_cols"]], I16,
                            kind="ExternalInput")
    sidx_d = nc.dram_tensor("sidx", [P, common["sidx_cols"]], I16,
                            kind="ExternalInput")
    deg_d = nc.dram_tensor("deg", [P, TB], I32, kind="ExternalInput")
    w1_d = nc.dram_tensor("w1", [DIN, 2 * H], F32, kind="ExternalInput")
    w2a_d = nc.dram_tensor("w2a", [H, H2], F32, kind="ExternalInput")
    w2b_d = nc.dram_tensor("w2b", [H, H2], F32, kind="ExternalInput")
    wlin_d = nc.dram_tensor("wlin", [H2 + 1, NOUT], F32, kind="ExternalInput")
    b1b_d = nc.dram_tensor("b1b", [P, H], F32, kind="ExternalInput")
    b2c_d = nc.dram_tensor("b2c", [H2, 1], F32, kind="ExternalInput")
    out_d = nc.dram_tensor("out", [NOUT, NLOC], F32, kind="ExternalOutput")

    # internal DRAM
    y1tab = nc.dram_tensor("y1tab", [TBL, H], F32)
    h1tab = nc.dram_tensor("h1tab", [TBL, H], F32)
    partial1 = nc.dram_tensor("partial1", [NPG, H], F32)
    partial2 = nc.dram_tensor("partial2", [NPG, H], F32)
    rs1 = nc.dram_tensor("rs1", [NLOC, H], F32)
    rs2 = nc.dram_tensor("rs2", [NLOC, H], F32)

    groups = [list(range(N_CORES))]

    with tile.TileContext(nc) as tc:
        with (
            tc.tile_pool(name="const", bufs=1) as kpool,
            tc.tile_pool(name="work", bufs=2) as wpool,
            tc.tile_pool(name="small", bufs=4) as mpool,
            tc.tile_pool(name="gather",
                         bufs=int(os.environ.get("KGBUFS", "2"))) as gpool,
            tc.tile_pool(name="cidx", bufs=2) as cpool,
            tc.tile_pool(name="stage",
                         bufs=int(os.environ.get("KSBUFS", "4"))) as spool,
            tc.tile_pool(name="psA", bufs=2, space="PSUM") as psA,
            tc.tile_pool(name="psT", bufs=2, space="PSUM") as psT,
            tc.tile_pool(name="ps2", bufs=2, space="PSUM") as ps2,
            tc.tile_pool(name="ps3", bufs=2, space="PSUM") as ps3,
        ):
            pools = dict(gather=gpool, cidx=cpool, stage=spool)

            # ---- constants ----
            w1t = kpool.tile([DIN, 2 * H], F32)
            nc.sync.dma_start(out=w1t[:], in_=w1_d[:])
            w2at_f = kpool.tile([H, H2], F32)
            nc.sync.dma_start(out=w2at_f[:], in_=w2a_d[:])
            w2bt_f = kpool.tile([H, H2], F32)
            nc.sync.dma_start(out=w2bt_f[:], in_=w2b_d[:])
            w2at = kpool.tile([H, H2], BF16)
            nc.vector.tensor_copy(out=w2at[:], in_=w2at_f[:])
            w2bt = kpool.tile([H, H2], BF16)
            nc.vector.tensor_copy(out=w2bt[:], in_=w2bt_f[:])
            wlint = kpool.tile([H2 + 1, NOUT], F32)
            nc.sync.dma_start(out=wlint[:], in_=wlin_d[:])
            b1bt = kpool.tile([P, H], F32)
            nc.sync.dma_start(out=b1bt[:], in_=b1b_d[:])
            b2ct = kpool.tile([H2, 1], F32)
            nc.sync.dma_start(out=b2ct[:], in_=b2c_d[:])
            ident = kpool.tile([P, P], F32)
            make_identity(nc, ident[:])
            sidx_t = kpool.tile([P, common["sidx_cols"]], I16)
            nc.sync.dma_start(out=sidx_t[:], in_=sidx_d[:])
            h1T = kpool.tile([H, NLOC], BF16)
            selfbig = kpool.tile([P, TB * H], F32)
            zbig = kpool.tile([P, TB * NOUT], F32)
            ebig = kpool.tile([P, TB * NOUT], F32)
            mt = kpool.tile([P, TB], F32)
            stt = kpool.tile([P, TB], F32)
            lst = kpool.tile([P, TB], F32)

            degt = kpool.tile([P, TB], I32)
            nc.sync.dma_start(out=degt[:], in_=deg_d[:])
            dinv = kpool.tile([P, TB], F32)
            nc.vector.tensor_copy(out=dinv[:], in_=degt[:])
            nc.vector.tensor_scalar_max(dinv[:], dinv[:], 1.0)
            nc.vector.reciprocal(out=dinv[:], in_=dinv[:])

            ZB = 16
            assert NPG % ZB == 0
            ztile = kpool.tile([P, ZB * H], F32)
            nc.vector.memset(ztile[:], 0.0)
            zt = kpool.tile([P, H], F32)
            nc.vector.memset(zt[:], 0.0)

            # KREPS: unroll the whole computation to amortize dispatch
            # noise in wall-clock timing (perf experiments only; default 1)
            def zero_partial(part):
                view = part.ap().rearrange("(a b) d -> a (b d)", b=ZB)
                rows = view.shape[0]
                for r0 in range(0, rows, P):
                    r1 = min(r0 + P, rows)
                    nc.sync.dma_start(out=view[r0:r1, :],
                                      in_=ztile[:r1 - r0, :])

            for rep in range(int(os.environ.get("KREPS", "1"))):
                # ---- phase A: y1 = x@W1_l.T -> y1tab;
                #      selfbig = x@W1_r.T + b1 (SBUF-resident) ----
                sc_A, _ = nc.enter_named_scope("phaseA", False)
                for b0 in range(0, TB2, BBLK):
                    nb = min(BBLK, TB2 - b0)
                    xblk = wpool.tile([DIN, BBLK * P], F32, tag="xblk")
                    nc.sync.dma_start(out=xblk[:, :nb * P],
                                      in_=xt_d[:, b0 * P:(b0 + nb) * P])
                    yblk = wpool.tile([P, BBLK, H], F32, tag="yblk")
                    for i in range(nb):
                        t = b0 + i
                        ps = psA.tile([P, 2 * H], F32, tag="psA")
                        nc.tensor.matmul(out=ps[:],
                                         lhsT=xblk[:, i * P:(i + 1) * P],
                                         rhs=w1t[:], start=True, stop=True)
                        nc.scalar.activation(
                            yblk[:, i, :], ps[:, :H],
                            mybir.ActivationFunctionType.Copy)
                        if t < TB:
                            nc.vector.tensor_add(
                                out=selfbig[:, t * H:(t + 1) * H],
                                in0=ps[:, H:], in1=b1bt[:])
                    yv = y1tab[b0 * P:(b0 + nb) * P, :].rearrange(
                        "(t p) d -> p t d", p=P)
                    nc.sync.dma_start(out=yv, in_=yblk[:, :nb, :])
                nc.leave_named_scope("phaseA", sc_A, False)
                zero_partial(partial1)

                # ---- layer 1 aggregate + pipelined RS ----
                if KSTAGE >= 2:
                    sc_g1, _ = nc.enter_named_scope("L1gather", False)
                    _emit_gather_phase(nc, tc, pools, common, y1tab, partial1,
                                       rs1, gidx_d, sidx_t, tag="L1",
                                       groups=groups, rep=rep,
                                       nqueues=KQUEUES)
                    nc.leave_named_scope("L1gather", sc_g1, False)

                # ---- layer 1 post: h1 = relu(rs1*dinv + self) ----
                sc_p1, _ = nc.enter_named_scope("L1post", False)
                nc.sync.dma_start(out=h1tab[NLOC:TBL, :], in_=zt[:])
                for b0 in range(0, TB if KSTAGE >= 4 else 0, BBLK):
                    nb = min(BBLK, TB - b0)
                    rblk = wpool.tile([P, BBLK, H], F32, tag="rblk")
                    rv = rs1[b0 * P:(b0 + nb) * P, :].rearrange(
                        "(t p) d -> p t d", p=P)
                    nc.sync.dma_start(out=rblk[:, :nb, :], in_=rv)
                    hblk = wpool.tile([P, BBLK, H], F32, tag="hblk")
                    for i in range(nb):
                        t = b0 + i
                        nc.vector.scalar_tensor_tensor(
                            out=hblk[:, i, :], in0=rblk[:, i, :],
                            scalar=dinv[:, t:t + 1],
                            in1=selfbig[:, t * H:(t + 1) * H],
                            op0=mybir.AluOpType.mult,
                            op1=mybir.AluOpType.add)
                        nc.vector.tensor_scalar_max(hblk[:, i, :],
                                                    hblk[:, i, :], 0.0)
                        pst = psT.tile([H, P], F32, tag="psT")
                        nc.tensor.transpose(out=pst[:], in_=hblk[:, i, :],
                                            identity=ident[:])
                        nc.scalar.activation(
                            h1T[:, t * P:(t + 1) * P], pst[:],
                            mybir.ActivationFunctionType.Copy)
                    hv = h1tab[b0 * P:(b0 + nb) * P, :].rearrange(
                        "(t p) d -> p t d", p=P)
                    nc.sync.dma_start(out=hv, in_=hblk[:, :nb, :])
                nc.leave_named_scope("L1post", sc_p1, False)
                zero_partial(partial2)

                # ---- layer 2 aggregate + pipelined RS ----
                if KSTAGE >= 5:
                    sc_g2, _ = nc.enter_named_scope("L2gather", False)
                    _emit_gather_phase(nc, tc, pools, common, h1tab, partial2,
                                       rs2, gidx_d, sidx_t, tag="L2",
                                       groups=groups, rep=rep,
                                       nqueues=KQUEUES)
                    nc.leave_named_scope("L2gather", sc_g2, False)

                # ---- layer 2 post + head ----
                sc_p2, _ = nc.enter_named_scope("L2post", False)
                for b0 in range(0, TB if KSTAGE >= 6 else 0, BBLK):
                    nb = min(BBLK, TB - b0)
                    rblk = wpool.tile([P, BBLK, H], F32, tag="rblk2")
                    rv = rs2[b0 * P:(b0 + nb) * P, :].rearrange(
                        "(t p) d -> p t d", p=P)
                    nc.sync.dma_start(out=rblk[:, :nb, :], in_=rv)
                    for i in range(nb):
                        t = b0 + i
                        a32 = mpool.tile([P, H], F32, tag="a32")
                        nc.vector.tensor_scalar_mul(a32[:], rblk[:, i, :],
                                                    dinv[:, t:t + 1])
                        pst = psT.tile([H, P], F32, tag="psT")
                        nc.tensor.transpose(out=pst[:], in_=a32[:],
                                            identity=ident[:])
                        aggT = mpool.tile([H, P], BF16, tag="aggT")
                        nc.scalar.activation(
                            aggT[:], pst[:],
                            mybir.ActivationFunctionType.Copy)
                        p2 = ps2.tile([H2, P], F32, tag="p2")
                        nc.tensor.matmul(out=p2[:], lhsT=w2at[:], rhs=aggT[:],
                                         start=True, stop=False)
                        nc.tensor.matmul(out=p2[:], lhsT=w2bt[:],
                                         rhs=h1T[:, t * P:(t + 1) * P],
                                         start=False, stop=True)
                        h2T = mpool.tile([H2 + 1, P], F32, tag="h2T")
                        nc.vector.tensor_scalar(
                            out=h2T[:H2, :], in0=p2[:],
                            scalar1=b2ct[:, :1], scalar2=0.0,
                            op0=mybir.AluOpType.add,
                            op1=mybir.AluOpType.max)
                        nc.vector.memset(h2T[H2:H2 + 1, :], 1.0)
                        # logits = h2 @ W_lin.T + b_lin (ones-row folds bias)
                        p3 = ps3.tile([P, NOUT], F32, tag="p3")
                        nc.tensor.matmul(out=p3[:], lhsT=h2T[:], rhs=wlint[:],
                                         start=True, stop=True)
                        nc.scalar.activation(
                            zbig[:, t * NOUT:(t + 1) * NOUT], p3[:],
                            mybir.ActivationFunctionType.Copy)
                if KSTAGE >= 6:
                    # ---- batched log-softmax over the 2 classes ----
                    zv = zbig[:].rearrange("p (t c) -> p t c", c=NOUT)
                    ev = ebig[:].rearrange("p (t c) -> p t c", c=NOUT)
                    nc.vector.tensor_max(out=mt[:], in0=zv[:, :, 0],
                                         in1=zv[:, :, 1])
                    nc.vector.tensor_sub(out=zv[:, :, 0], in0=zv[:, :, 0],
                                         in1=mt[:])
                    nc.vector.tensor_sub(out=zv[:, :, 1], in0=zv[:, :, 1],
                                         in1=mt[:])
                    nc.scalar.activation(ebig[:], zbig[:],
                                         mybir.ActivationFunctionType.Exp)
                    nc.vector.tensor_add(out=stt[:], in0=ev[:, :, 0],
                                         in1=ev[:, :, 1])
                    nc.scalar.activation(lst[:], stt[:],
                                         mybir.ActivationFunctionType.Ln)
                    nc.vector.tensor_sub(out=zv[:, :, 0], in0=zv[:, :, 0],
                                         in1=lst[:])
                    nc.vector.tensor_sub(out=zv[:, :, 1], in0=zv[:, :, 1],
                                         in1=lst[:])
                    # ---- transpose out to [2, NLOC] and store ----
                    for b0 in range(0, TB, BBLK):
                        nb = min(BBLK, TB - b0)
                        rT = wpool.tile([NOUT, BBLK * P], F32, tag="rT")
                        for i in range(nb):
                            t = b0 + i
                            pst = psT.tile([H, P], F32, tag="psT")
                            nc.tensor.transpose(
                                out=pst[:NOUT, :],
                                in_=zbig[:, t * NOUT:(t + 1) * NOUT],
                                identity=ident[:])
                            nc.vector.tensor_copy(
                                out=rT[:, i * P:(i + 1) * P],
                                in_=pst[:NOUT, :])
                        nc.sync.dma_start(
                            out=out_d[:, b0 * P:(b0 + nb) * P],
                            in_=rT[:, :nb * P])
                nc.leave_named_scope("L2post", sc_p2, False)

    nc.compile()
    return nc


# ----------------------------------------------------------------------------
# runner: persistent jitted executable (mirrors bass2jax.run_bass_via_pjrt,
# but reusable so repeat executions can be wall-clock timed)
# ----------------------------------------------------------------------------

def make_runner(nc, n_cores=N_CORES):
    import jax
    from jax.sharding import Mesh, PartitionSpec
    from jax.experimental.shard_map import shard_map
    import concourse.mybir as mb
    from concourse import bass2jax

    bass2jax.install_neuronx_cc_hook()
    assert nc.dbg_addr is None
    pname = nc.partition_id_tensor.name if nc.partition_id_tensor else None

    in_names, out_names, out_avals = [], [], []
    for alloc in nc.m.functions[0].allocations:
        if not isinstance(alloc, mb.MemoryLocationSet):
            continue
        name = alloc.memorylocations[0].name
        if alloc.kind == "ExternalInput":
            if name != pname:
                in_names.append(name)
        elif alloc.kind == "ExternalOutput":
            out_names.append(name)
            out_avals.append(jax.core.ShapedArray(
                tuple(alloc.tensor_shape), mb.dt.np(alloc.dtype)))
    n_params = len(in_names)
    all_names = in_names + out_names
    if pname is not None:
        all_names = all_names + [pname]

    def _body(*args):
        operands = list(args)
        if pname is not None:
            operands.append(bass2jax.partition_id_tensor())
        outs = bass2jax._bass_exec_p.bind(
            *operands, out_avals=tuple(out_avals), in_names=tuple(all_names),
            out_names=tuple(out_names), lowering_input_output_aliases=(),
            sim_require_finite=True, sim_require_nnan=True, nc=nc)
        return tuple(outs)

    devices = jax.devices()[:n_cores]
    mesh = Mesh(np.asarray(devices), ("core",))
    n_outs = len(out_names)
    sharded = jax.jit(
        shard_map(_body, mesh=mesh,
                  in_specs=(PartitionSpec("core"),) * (n_params + n_outs),
                  out_specs=(PartitionSpec("core"),) * n_outs,
                  check_rep=False),
        donate_argnums=tuple(range(n_params, n_params + n_outs)),
        keep_unused=True)

    from jax.sharding import NamedSharding
    shard = NamedSharding(mesh, PartitionSpec("core"))

    def prepare(in_maps):
        """Pre-stage the concatenated inputs on the devices."""
        concat_in = [np.concatenate([np.asarray(m[nm]) for m in in_maps],
                                    axis=0) for nm in in_names]
        dev_in = [jax.device_put(a, shard) for a in concat_in]
        jax.block_until_ready(dev_in)
        return dev_in

    def run_prepared(dev_in):
        concat_zeros = [np.zeros((n_cores * a.shape[0], *a.shape[1:]),
                                 a.dtype) for a in out_avals]
        dev_zeros = [jax.device_put(z, shard) for z in concat_zeros]
        jax.block_until_ready(dev_zeros)
        outs = sharded(*dev_in, *dev_zeros)
        return jax.block_until_ready(outs)

    def run(in_maps):
        outs = run_prepared(prepare(in_maps))
        return [
            {nm: np.asarray(outs[i]).reshape(n_cores, *out_avals[i].shape)[c]
             for i, nm in enumerate(out_names)}
            for c in range(n_cores)
        ]

    run.prepare = prepare
    run.run_prepared = run_prepared
    return run


# ----------------------------------------------------------------------------
# entry point
# ----------------------------------------------------------------------------

def kernel(x, edge_index, W1_l, b1_l, W1_r, W2_l, b2_l, W2_r, W_lin, b_lin):
    x = np.ascontiguousarray(np.asarray(x, np.float32))
    ei = np.asarray(edge_index)
    src = ei[0].astype(np.int64)
    dst = ei[1].astype(np.int64)
    n_nodes = x.shape[0]

    weights = (np.asarray(W1_l, np.float32), np.asarray(b1_l, np.float32),
               np.asarray(W1_r, np.float32), np.asarray(W2_l, np.float32),
               np.asarray(b2_l, np.float32), np.asarray(W2_r, np.float32),
               np.asarray(W_lin, np.float32), np.asarray(b_lin, np.float32))

    common, per_core, deg_global = _build_structure(src, dst, n_nodes)
    in_maps = [_build_core_inputs(common, per_core[k], deg_global, k, x,
                                  weights) for k in range(N_CORES)]

    nc = _build_program(common)

    run = make_runner(nc)
    results = run(in_maps)
    LAST_INFO.clear()
    LAST_INFO.update(slot_tot=common["slot_tot"], runner=run,
                     in_maps=in_maps, nc=nc)

    N, S = common["N"], common["S"]
    CHUNK, NLOC = common["CHUNK"], common["NLOC"]
    out = np.zeros((N, NOUT), np.float32)
    for k in range(N_CORES):
        res = results[k]["out"].T                          # [NLOC, NOUT]
        own = _owned_nodes(k, N, S, CHUNK, NLOC)
        v = own >= 0
        out[own[v]] = res[v]
    return out.astype(np.float32)
